# revision 13
# baseline (speedup 1.0000x reference)
"""Trainium2 Bass kernel for nn_DeepBSDESC (DeepBSDE forward pass).

Strategy
--------
The reference scan over 128 time steps is *affine* in the carried state u:
    u_{k+1} = c_k * u_k + a_k
where c_k (real) and a_k (complex) do not depend on u.  Hence
    u_final = (prod_k c_k) * u0 + sum_k a_k * prod_{j>k} c_j
and every step's a_k can be evaluated independently (no sequential loop on
device).  The 3x3 matrix algebra collapses analytically:
    T_inv @ sigma0^T = 0.5*I - 0.4*x x^T / (1+|x|^2)
so grad_bmm reduces to dot products.

Sharding: data-parallel over batch B=32768 across 8 cores (4096 each), MLP
weights replicated.  Host precomputes input-only coefficient planes (masks,
suffix products, exp-functional phases); the device evaluates all MLPs
(>99% of FLOPs) and the per-step combine, then reduces over steps.

Device pipeline per core, per 512-batch chunk, per step k:
  L1  : z = [ft @ (Wg1|Wj1) + bias ; ft @ (Wj1|Wj1) +- w0-shifted bias]
        via two f32r matmuls (bias through an appended ones-feature row)
  tanh: one ACT pass [128,1024] PSUM->SBUF fp16
  L2  : transposed matmuls (H chunk stationary, small weight matrix moving)
        -> batch-major outputs accumulated in PSUM over 32 steps
  combine: DVE elementwise with host coefficient planes, reduce over steps.
"""

import os
import sys

import numpy as np

for _p in ("/opt/trn_rl_repo", "/root/.axon_site/_ro/trn_rl_repo"):
    if os.path.isdir(_p) and _p not in sys.path:
        sys.path.append(_p)

from contextlib import ExitStack

import concourse.bass as bass
import concourse.bacc as bacc
import concourse.tile as tile
from concourse import mybir
from concourse.bass_utils import run_bass_kernel_spmd

N_CORES = 8
NK = 128                 # time steps
B_FULL = 32768
B_LOC = B_FULL // N_CORES  # 4096
NBC = 8                  # 512-batch chunks per core
BC = 512
DT_STEP = 1.0 / NK

F32 = mybir.dt.float32
F32R = mybir.dt.float32r
F16 = mybir.dt.float16
AF = mybir.ActivationFunctionType
AX = mybir.AxisListType

# plane blob column offsets (per 512-batch chunk, [128, 7680])
_XP, _DBP = 0, 1536
_E1, _E2, _CUP, _CUM, _QP, _EFPR, _EFPI = 3072, 3584, 4096, 4608, 5120, 5632, 6144
_DCR, _DCI = 6656, 7168
PL_COLS = 7680


def _phase_a(nc, tc, pools, bc, ft_d, w1_d, w2all, w1r, w2r, oall, U0):
    """MLP evaluation for one 512-batch chunk, all 128 steps + u0."""
    p_ft, p_w1, p_h, p_zps, p_ops = pools
    ftt = None
    for kg in range(4):
        ops_tiles = [p_ops.tile([128, 384], F32, tag="ops", name=f"ops{c}")
                     for c in range(4)]
        for kk in range(32):
            k = kg * 32 + kk
            if k % 16 == 0:
                w1t = p_w1.tile([6, 16 * 256], F32R, tag="w1")
                nc.sync.dma_start(
                    out=w1t[:].rearrange("p (a b) -> p a b", a=16),
                    in_=w1_d[k:k + 16].rearrange("a p b -> p a b"),
                )
            if k % 8 == 0:
                ftt = p_ft.tile([6, 8 * BC], F32R, tag="ft")
                nc.sync.dma_start(
                    out=ftt[:].rearrange("p (a b) -> p a b", a=8),
                    in_=ft_d[bc, :, k:k + 8, :],
                )
            zt = p_zps.tile([128, 1024], F32, tag="z")
            rhs = ftt[:, (k % 8) * BC:(k % 8 + 1) * BC]
            kw = kk % 16
            nc.tensor.matmul(zt[:, 0:512], w1t[:, kw * 256:kw * 256 + 128], rhs,
                             start=True, stop=True)
            nc.tensor.matmul(zt[:, 512:1024], w1t[:, kw * 256 + 128:kw * 256 + 256],
                             rhs, start=True, stop=True)
            ht = p_h.tile([128, 1024], F16, tag="h")
            nc.scalar.activation(ht[:], zt[:], AF.Tanh)
            for c in range(4):
                nc.tensor.matmul(
                    ops_tiles[c][:, kk * 12:kk * 12 + 8],
                    ht[:, c * 128:(c + 1) * 128],
                    w2all[:, k * 12:k * 12 + 8],
                    start=True, stop=True,
                )
                nc.tensor.matmul(
                    ops_tiles[c][:, kk * 12 + 8:kk * 12 + 12],
                    ht[:, 512 + c * 128:512 + (c + 1) * 128],
                    w2all[:, k * 12 + 8:k * 12 + 12],
                    start=True, stop=True,
                )
        for c in range(4):
            nc.vector.tensor_copy(
                oall[:, c * 1536 + kg * 384:c * 1536 + (kg + 1) * 384],
                ops_tiles[c][:],
            )
    # u0 MLP (reference-head weights, feat at k=0)
    ft0 = p_w1.tile([6, 512], F32R, tag="ft0")
    nc.sync.dma_start(out=ft0[:], in_=ft_d[bc, :, 0, :])
    z0 = p_zps.tile([128, 1024], F32, tag="z")
    nc.tensor.matmul(z0[:, 0:512], w1r[:], ft0[:], start=True, stop=True)
    h0 = p_h.tile([128, 1024], F16, tag="h")
    nc.scalar.activation(h0[:, 0:512], z0[:, 0:512], AF.Tanh)
    ou = p_ops.tile([128, 384], F32, tag="ops")
    for c in range(4):
        nc.tensor.matmul(ou[:, c * 2:(c + 1) * 2], h0[:, c * 128:(c + 1) * 128],
                         w2r[:], start=True, stop=True)
    nc.vector.tensor_copy(U0[:, bc * 8:(bc + 1) * 8], ou[:, 0:8])


def _phase_b(nc, tc, p_tmp, bc, pl, oall, Sr, Si):
    """Elementwise combine + step reduction for one 512-batch chunk."""
    o4 = oall[:].rearrange("p (c k j) -> p c k j", c=4, k=NK)
    og_r, og_i = o4[:, :, :, 0:3], o4[:, :, :, 3:6]
    oi_r, oi_i = o4[:, :, :, 6], o4[:, :, :, 7]
    op_r, op_i = o4[:, :, :, 8], o4[:, :, :, 9]
    om_r, om_i = o4[:, :, :, 10], o4[:, :, :, 11]
    xp = pl[:, _XP:_XP + 1536].rearrange("p (c k i) -> p c k i", c=4, k=NK)
    dbp = pl[:, _DBP:_DBP + 1536].rearrange("p (c k i) -> p c k i", c=4, k=NK)
    E1 = pl[:, _E1:_E1 + 512]
    E2 = pl[:, _E2:_E2 + 512]
    CUP = pl[:, _CUP:_CUP + 512]
    CUM = pl[:, _CUM:_CUM + 512]
    QP = pl[:, _QP:_QP + 512]
    EFPR = pl[:, _EFPR:_EFPR + 512]
    EFPI = pl[:, _EFPI:_EFPI + 512]
    DCR = pl[:, _DCR:_DCR + 512]
    DCI = pl[:, _DCI:_DCI + 512]

    def T(name, cols=512):
        return p_tmp.tile([128, cols], F32, tag=name, name=name)[:]

    prod = p_tmp.tile([128, 1536], F32, tag="prod", name="prod")[:]
    prod4 = prod.rearrange("p (c k i) -> p c k i", c=4, k=NK)

    v = nc.vector
    d1r, d1i, d2r, d2i = T("d1r"), T("d1i"), T("d2r"), T("d2i")
    sgr, sgi = T("sgr"), T("sgi")
    dupr, dupi, dumr, dumi = T("dupr"), T("dupi"), T("dumr"), T("dumi")
    deltr, delti = T("deltr"), T("delti")
    apr, api = T("apr"), T("api")
    t1 = T("t1")

    for (dst, a, b_) in ((d1r, og_r, dbp), (d1i, og_i, dbp),
                         (d2r, og_r, xp), (d2i, og_i, xp)):
        v.tensor_mul(prod4, a, b_)
        v.reduce_sum(dst, prod4, axis=AX.X)
    v.reduce_sum(sgr, og_r, axis=AX.X)
    v.reduce_sum(sgi, og_i, axis=AX.X)
    v.tensor_sub(dupr, op_r, oi_r)
    v.tensor_sub(dupi, op_i, oi_i)
    v.tensor_sub(dumr, om_r, oi_r)
    v.tensor_sub(dumi, om_i, oi_i)

    for (dst, dd1, dd2, dup, dum, sg, dc) in (
            (deltr, d1r, d2r, dupr, dumr, sgr, DCR),
            (delti, d1i, d2i, dupi, dumi, sgi, DCI)):
        v.tensor_mul(dst, E1, dd1)
        v.tensor_mul(t1, E2, dd2)
        v.tensor_sub(dst, dst, t1)
        v.tensor_mul(t1, CUP, dup)
        v.tensor_add(dst, dst, t1)
        v.tensor_mul(t1, CUM, dum)
        v.tensor_add(dst, dst, t1)
        v.tensor_mul(t1, QP, sg)
        v.tensor_sub(dst, dst, t1)
        v.tensor_add(dst, dst, dc)

    v.tensor_mul(apr, EFPR, deltr)
    v.tensor_mul(t1, EFPI, delti)
    v.tensor_sub(apr, apr, t1)
    v.tensor_mul(api, EFPR, delti)
    v.tensor_mul(t1, EFPI, deltr)
    v.tensor_add(api, api, t1)

    v.reduce_sum(Sr[:, bc * 4:(bc + 1) * 4],
                 apr.rearrange("p (c k) -> p c k", c=4), axis=AX.X)
    v.reduce_sum(Si[:, bc * 4:(bc + 1) * 4],
                 api.rearrange("p (c k) -> p c k", c=4), axis=AX.X)


def _kernel_body(ctx, tc, ft_d, w1_d, w2_d, w1r_d, w2r_d, pl_d, fin_d, u_d, g_d):
    nc = tc.nc
    p_const = ctx.enter_context(tc.tile_pool(name="const", bufs=1))
    p_ft = ctx.enter_context(tc.tile_pool(name="ftp", bufs=2))
    p_w1 = ctx.enter_context(tc.tile_pool(name="w1p", bufs=2))
    p_h = ctx.enter_context(tc.tile_pool(name="hp", bufs=2))
    p_oall = ctx.enter_context(tc.tile_pool(name="oallp", bufs=2))
    p_pl = ctx.enter_context(tc.tile_pool(name="plp", bufs=1))
    p_tmp = ctx.enter_context(tc.tile_pool(name="tmpp", bufs=1))
    p_zps = ctx.enter_context(tc.tile_pool(name="zpsp", bufs=2, space="PSUM"))
    p_ops = ctx.enter_context(tc.tile_pool(name="opsp", bufs=4, space="PSUM"))

    w2all = p_const.tile([128, NK * 12], F16)
    nc.sync.dma_start(out=w2all[:].rearrange("p (k j) -> p k j", k=NK), in_=w2_d)
    w1r = p_const.tile([6, 128], F32R)
    nc.sync.dma_start(out=w1r[:], in_=w1r_d)
    w2r = p_const.tile([128, 2], F16)
    nc.sync.dma_start(out=w2r[:], in_=w2r_d)
    fin = p_const.tile([128, 192], F32)
    nc.sync.dma_start(out=fin[:], in_=fin_d)

    Sr = p_const.tile([128, 4 * NBC], F32)
    Si = p_const.tile([128, 4 * NBC], F32)
    U0 = p_const.tile([128, 8 * NBC], F32)

    mlp_pools = (p_ft, p_w1, p_h, p_zps, p_ops)
    for bc in range(NBC):
        pl = p_pl.tile([128, PL_COLS], F32, tag="pl")
        nc.gpsimd.dma_start(out=pl[:], in_=pl_d[bc])
        oall_t = p_oall.tile([128, 6144], F16, tag="oall")
        _phase_a(nc, tc, mlp_pools, bc, ft_d, w1_d, w2all, w1r, w2r, oall_t[:], U0)
        _phase_b(nc, tc, p_tmp, bc, pl[:], oall_t[:], Sr[:], Si[:])

    # final assembly
    v = nc.vector
    PF = fin[:, 0:32]
    E8R = fin[:, 32:64]
    E8I = fin[:, 64:96]
    X1f, X2f, X3f = fin[:, 96:128], fin[:, 128:160], fin[:, 160:192]
    outu = p_const.tile([128, 8 * NBC], F32)
    outg = p_const.tile([128, 8 * NBC], F32)
    xs = p_tmp.tile([128, 32], F32, tag="fxs", name="fxs")[:]
    tu = p_tmp.tile([128, 32], F32, tag="ftu", name="ftu")[:]
    outu_v = outu[:].rearrange("p (b r) -> p b r", r=2)
    outg_v = outg[:].rearrange("p (b r) -> p b r", r=2)
    u0_v = U0[:].rearrange("p (b r) -> p b r", r=2)

    v.tensor_add(xs, X1f, X2f)
    v.tensor_add(xs, xs, X3f)
    v.tensor_mul(outg_v[:, :, 0], E8R, xs)
    v.tensor_mul(outg_v[:, :, 1], E8I, xs)
    v.tensor_mul(tu, u0_v[:, :, 0], PF)
    v.tensor_add(outu_v[:, :, 0], tu, Sr[:])
    v.tensor_mul(tu, u0_v[:, :, 1], PF)
    v.tensor_add(outu_v[:, :, 1], tu, Si[:])

    nc.sync.dma_start(
        out=u_d.rearrange("(bc c bp) ri -> bp bc c ri", bc=NBC, c=4),
        in_=outu[:].rearrange("p (bc c ri) -> p bc c ri", bc=NBC, c=4),
    )
    nc.sync.dma_start(
        out=g_d.rearrange("(bc c bp) ri -> bp bc c ri", bc=NBC, c=4),
        in_=outg[:].rearrange("p (bc c ri) -> p bc c ri", bc=NBC, c=4),
    )


def build_nc():
    nc = bacc.Bacc("TRN2", target_bir_lowering=False, debug=False)
    ft_d = nc.dram_tensor("ft", [NBC, 6, NK, BC], F32R, kind="ExternalInput").ap()
    w1_d = nc.dram_tensor("w1", [NK, 6, 256], F32R, kind="ExternalInput").ap()
    w2_d = nc.dram_tensor("w2", [128, NK, 12], F16, kind="ExternalInput").ap()
    w1r_d = nc.dram_tensor("w1r", [6, 128], F32R, kind="ExternalInput").ap()
    w2r_d = nc.dram_tensor("w2r", [128, 2], F16, kind="ExternalInput").ap()
    pl_d = nc.dram_tensor("planes", [NBC, 128, PL_COLS], F32, kind="ExternalInput").ap()
    fin_d = nc.dram_tensor("fin", [128, 192], F32, kind="ExternalInput").ap()
    u_d = nc.dram_tensor("u_ri", [B_LOC, 2], F32, kind="ExternalOutput").ap()
    g_d = nc.dram_tensor("g_ri", [B_LOC, 2], F32, kind="ExternalOutput").ap()
    with tile.TileContext(nc) as tc:
        with ExitStack() as ctx:
            _kernel_body(ctx, tc, ft_d, w1_d, w2_d, w1r_d, w2r_d, pl_d, fin_d,
                         u_d, g_d)
    nc.compile()
    return nc


# ----------------------------------------------------------------------------
# host-side preparation
# ----------------------------------------------------------------------------

def _to_bck(gc):
    """[128k, 4096b] -> [8bc, 128bp, 4c, 128k] for one core."""
    return gc.reshape(NK, NBC, 4, 128).transpose(1, 3, 2, 0)


def prep_host(inp):
    f32, f64 = np.float32, np.float64
    N = np.asarray(inp["process_N"], f32)[:, :, 0]
    X = np.asarray(inp["process_X"], f32)
    P = np.asarray(inp["discrete_p"], f32)[:, :, 0]
    T = np.asarray(inp["discrete_t"], f32)
    dB = np.asarray(inp["delta_B"], f32)

    n, x, p, t = N[:NK], X[:NK], P[:NK], T[:NK]
    dN = np.round(N[1:] - N[:NK])

    s = np.sum(x * x, axis=-1)
    theta = (p * s).astype(f64)
    phi = (DT_STEP * (np.cumsum(theta, axis=0) - theta)).astype(f64)
    efr = np.cos(phi).astype(f32)
    efi = (-np.sin(phi)).astype(f32)

    kD = np.sqrt(1.0 + 0.2 * np.abs(n))
    m0 = (dN == 0).astype(f32)
    mp_ = (dN > 0).astype(f32)
    mm_ = (dN < 0).astype(f32)
    w2c = 0.4 / (1.0 + s)
    d3 = np.sum(x * dB, axis=-1)
    E1 = m0 * kD * np.float32(0.5)
    E2 = m0 * kD * w2c * d3
    CUP = mp_ - m0 * (0.5 * (n + 1.0)) * np.float32(DT_STEP)
    CUM = mm_ - m0 * (0.4 * np.abs(n) + 0.1) * np.float32(DT_STEP)
    QP = m0 * (np.float32(0.1 * DT_STEP) * (1.0 + t[:, None]))
    c = (1.0 - m0 * p * np.float32(DT_STEP)).astype(f64)
    SP = np.ones_like(c)
    SP[:-1] = np.cumprod(c[::-1], axis=0)[::-1][1:]
    Pfull = (c[0] * SP[0]).astype(f32)
    EFPR = (efr * SP).astype(f32)
    EFPI = (efi * SP).astype(f32)

    phi128 = DT_STEP * np.cumsum(theta, axis=0)[-1]
    EF128R = np.cos(phi128).astype(f32)
    EF128I = (-np.sin(phi128)).astype(f32)

    # weights (shared across cores)
    Wg1, bg1 = np.asarray(inp["Wg1"], f32), np.asarray(inp["bg1"], f32)
    Wg2 = np.asarray(inp["Wg2"], f32)
    bg2 = np.asarray(inp["bg2"], f32)
    Wj1, bj1 = np.asarray(inp["Wj1"], f32), np.asarray(inp["bj1"], f32)
    Wj2 = np.asarray(inp["Wj2"], f32)
    bj2 = np.asarray(inp["bj2"], f32)
    Wr1, br1 = np.asarray(inp["Wr1"], f32), np.asarray(inp["br1"], f32)
    Wr2, br2 = np.asarray(inp["Wr2"], f32), np.asarray(inp["br2"], f32)
    w0 = Wj1[:, 0]

    w1a = np.zeros((NK, 6, 128), f32)
    w1a[:, 0:5, 0:64] = Wg1
    w1a[:, 5, 0:64] = bg1
    w1a[:, 0:5, 64:128] = Wj1
    w1a[:, 5, 64:128] = bj1
    w1b = np.zeros((NK, 6, 128), f32)
    w1b[:, 0:5, 0:64] = Wj1
    w1b[:, 5, 0:64] = bj1 + w0
    w1b[:, 0:5, 64:128] = Wj1
    w1b[:, 5, 64:128] = bj1 - w0
    w1_host = np.ascontiguousarray(np.concatenate([w1a, w1b], axis=2))

    # The device layer-2 omits the output biases: dup/dum cancel bj2 exactly,
    # and the constant bg2 contribution to delt is folded into the additive
    # host planes DCR/DCI below.  br2 (u0 head) is re-added on the host.
    w2cat = np.zeros((NK, 128, 12), f32)
    w2cat[:, 0:64, 0:6] = Wg2
    w2cat[:, 64:128, 6:8] = Wj2
    w2cat[:, 0:64, 8:10] = Wj2
    w2cat[:, 64:128, 10:12] = Wj2
    w2_host = np.ascontiguousarray(w2cat.transpose(1, 0, 2)).astype(np.float16)

    w1r_host = np.zeros((6, 128), f32)
    w1r_host[0:5, 0:64] = Wr1
    w1r_host[5, 0:64] = br1
    w2r_host = np.zeros((128, 2), np.float16)
    w2r_host[0:64] = Wr2.astype(np.float16)

    # --- fold constant layer-2 biases into the coefficient planes ---
    # o_g = o_g_dev + bg2  (bg2 = [br(3), bi(3)])
    # o_i/o_p/o_m get + bj2, which cancels in dup/dum.
    # delt uses: E1*(gu.dB) - E2*(gu.x) - QP*sum(gu)  with gu = gu_dev + bg2.
    # Additive correction (real):  E1*(bgr.dB) - E2*(bgr.x) - QP*sum(bgr)
    # This is an input-only plane; append to EFP-side as a delt offset:
    #   deltr_true = deltr_dev + DCR ,  delti_true = delti_dev + DCI
    # Then a' = EFP * delt: fold DCR/DCI into S on the host?  S is a device
    # reduction of EFP*delt; the correction sum_k EFP_k*DC_k is fully
    # host-computable, so add it to u via the final combine: we fold it into
    # the PF/S path by adding the correction to Sr/Si through... the device
    # adds outu = u0*PF + S; host cannot inject there.  We instead fold DC
    # into the plane pair (EFPR, EFPI) is impossible (multiplicative).
    # => device-side: deltr starts as E1*d1 (dev) ... we add one more fused
    # add using a 14th/15th plane pair DCR/DCI.
    bgr, bgi = bg2[:, 0:3], bg2[:, 3:6]
    DCR = (E1 * np.einsum("kj,kbj->kb", bgr, dB)
           - E2 * np.einsum("kj,kbj->kb", bgr, x)
           - QP * bgr.sum(axis=1)[:, None])
    DCI = (E1 * np.einsum("kj,kbj->kb", bgi, dB)
           - E2 * np.einsum("kj,kbj->kb", bgi, x)
           - QP * bgi.sum(axis=1)[:, None])

    in_maps = []
    for ci in range(N_CORES):
        sl = slice(ci * B_LOC, (ci + 1) * B_LOC)
        ftc = np.stack([n[:, sl], x[:, sl, 0], x[:, sl, 1], x[:, sl, 2],
                        p[:, sl], np.ones_like(p[:, sl])], axis=1)  # [128,6,4096]
        ft_host = np.ascontiguousarray(
            ftc.reshape(NK, 6, NBC, BC).transpose(2, 1, 0, 3))

        xpc = X[:NK, sl].reshape(NK, NBC, 4, 128, 3).transpose(1, 3, 2, 0, 4)
        dbc = dB[:, sl].reshape(NK, NBC, 4, 128, 3).transpose(1, 3, 2, 0, 4)
        singles = [_to_bck(a[:, sl]) for a in
                   (E1, E2, CUP, CUM, QP, EFPR, EFPI, DCR, DCI)]
        pl_host = np.concatenate(
            [xpc.reshape(NBC, 128, 1536), dbc.reshape(NBC, 128, 1536)]
            + [a.reshape(NBC, 128, 512) for a in singles], axis=2)
        pl_host = np.ascontiguousarray(pl_host, dtype=f32)

        def fincol(a):
            return a[sl].reshape(NBC, 4, 128).transpose(2, 0, 1).reshape(128, 32)

        fin_host = np.ascontiguousarray(np.concatenate(
            [fincol(Pfull), fincol(EF128R), fincol(EF128I),
             fincol(X[NK, :, 0]), fincol(X[NK, :, 1]), fincol(X[NK, :, 2])],
            axis=1), dtype=f32)

        in_maps.append({
            "ft": ft_host, "w1": w1_host, "w2": w2_host,
            "w1r": w1r_host, "w2r": w2r_host,
            "planes": pl_host, "fin": fin_host,
        })
    return in_maps, Pfull, br2


_NC_CACHE = {}


def kernel(**inputs):
    in_maps, Pfull, br2 = prep_host(inputs)
    if "nc" not in _NC_CACHE:
        _NC_CACHE["nc"] = build_nc()
    nc = _NC_CACHE["nc"]
    res = run_bass_kernel_spmd(nc, in_maps, list(range(N_CORES)))
    u_parts, g_parts = [], []
    for ci in range(N_CORES):
        ur = res.results[ci]["u_ri"]
        gr = res.results[ci]["g_ri"]
        u_parts.append(ur[:, 0] + 1j * ur[:, 1])
        g_parts.append(gr[:, 0] + 1j * gr[:, 1])
    u = np.concatenate(u_parts)
    # fold the u0 layer-2 bias (br2, constant) back in: u += (br2_r+i br2_i)*Pfull
    u = u + (br2[0] + 1j * br2[1]) * Pfull.astype(np.float64)
    g = np.concatenate(g_parts)
    u = u.astype(np.complex64)[:, None]
    g = g.astype(np.complex64)[:, None]
    return u, g


# revision 14
# speedup vs baseline: 34.2364x; 34.2364x over previous
"""Trainium2 Bass kernel for nn_DeepBSDESC (DeepBSDE forward pass).

Strategy
--------
The reference scan over 128 time steps is *affine* in the carried state u:
    u_{k+1} = c_k * u_k + a_k
where c_k (real) and a_k (complex) do not depend on u.  Hence
    u_final = (prod_k c_k) * u0 + sum_k a_k * prod_{j>k} c_j
and every step's a_k can be evaluated independently (no sequential loop on
device).  The 3x3 matrix algebra collapses analytically:
    T_inv @ sigma0^T = 0.5*I - 0.4*x x^T / (1+|x|^2)
so grad_bmm reduces to dot products.

Sharding: data-parallel over batch B=32768 across 8 cores (4096 each), MLP
weights replicated.  Host precomputes input-only coefficient planes (masks,
suffix products, exp-functional phases); the device evaluates all MLPs
(>99% of FLOPs) and the per-step combine, then reduces over steps.

Device pipeline per core, per 512-batch chunk, per step k:
  L1  : z = [ft @ (Wg1|Wj1) + bias ; ft @ (Wj1|Wj1) +- w0-shifted bias]
        via two f32r matmuls (bias through an appended ones-feature row)
  tanh: one ACT pass [128,1024] PSUM->SBUF fp16
  L2  : transposed matmuls (H chunk stationary, small weight matrix moving)
        -> batch-major outputs accumulated in PSUM over 32 steps
  combine: DVE elementwise with host coefficient planes, reduce over steps.
"""

import os
import sys

import numpy as np

for _p in ("/opt/trn_rl_repo", "/root/.axon_site/_ro/trn_rl_repo"):
    if os.path.isdir(_p) and _p not in sys.path:
        sys.path.append(_p)

from contextlib import ExitStack

import concourse.bass as bass
import concourse.bacc as bacc
import concourse.tile as tile
from concourse import mybir
from concourse.bass_utils import run_bass_kernel_spmd

N_CORES = 8
NK = 128                 # time steps
B_FULL = 32768
B_LOC = B_FULL // N_CORES  # 4096
NBC = 8                  # 512-batch chunks per core
BC = 512
DT_STEP = 1.0 / NK

F32 = mybir.dt.float32
F32R = mybir.dt.float32r
F16 = mybir.dt.float16
AF = mybir.ActivationFunctionType
AX = mybir.AxisListType

# plane blob column offsets (per 512-batch chunk, [128, 7680])
_XP, _DBP = 0, 1536
_E1, _E2, _CUP, _CUM, _QP, _EFPR, _EFPI = 3072, 3584, 4096, 4608, 5120, 5632, 6144
_DCR, _DCI = 6656, 7168
PL_COLS = 7680


def _phase_a(nc, tc, pools, bc, ft_d, w1_d, w2all, w1r, w2r, oall, U0):
    """MLP evaluation for one 512-batch chunk, all 128 steps + u0."""
    p_ft, p_w1, p_h, p_zps, p_ops = pools
    ftt = None
    for kg in range(4):
        ops_tiles = [p_ops.tile([128, 384], F32, tag="ops", name=f"ops{c}")
                     for c in range(4)]
        for kk in range(32):
            k = kg * 32 + kk
            if k % 16 == 0:
                w1t = p_w1.tile([6, 16 * 256], F32R, tag="w1")
                nc.sync.dma_start(
                    out=w1t[:].rearrange("p (a b) -> p a b", a=16),
                    in_=w1_d[k:k + 16].rearrange("a p b -> p a b"),
                )
            if k % 8 == 0:
                ftt = p_ft.tile([6, 8 * BC], F32R, tag="ft")
                nc.sync.dma_start(
                    out=ftt[:].rearrange("p (a b) -> p a b", a=8),
                    in_=ft_d[bc, :, k:k + 8, :],
                )
            zt = p_zps.tile([128, 1024], F32, tag="z")
            rhs = ftt[:, (k % 8) * BC:(k % 8 + 1) * BC]
            kw = kk % 16
            nc.tensor.matmul(zt[:, 0:512], w1t[:, kw * 256:kw * 256 + 128], rhs,
                             start=True, stop=True)
            nc.tensor.matmul(zt[:, 512:1024], w1t[:, kw * 256 + 128:kw * 256 + 256],
                             rhs, start=True, stop=True)
            ht = p_h.tile([128, 1024], F16, tag="h")
            nc.scalar.activation(ht[:], zt[:], AF.Tanh)
            for c in range(4):
                nc.tensor.matmul(
                    ops_tiles[c][:, kk * 12:kk * 12 + 8],
                    ht[:, c * 128:(c + 1) * 128],
                    w2all[:, k * 12:k * 12 + 8],
                    start=True, stop=True,
                )
                nc.tensor.matmul(
                    ops_tiles[c][:, kk * 12 + 8:kk * 12 + 12],
                    ht[:, 512 + c * 128:512 + (c + 1) * 128],
                    w2all[:, k * 12 + 8:k * 12 + 12],
                    start=True, stop=True,
                )
        for c in range(4):
            nc.vector.tensor_copy(
                oall[:, c * 1536 + kg * 384:c * 1536 + (kg + 1) * 384],
                ops_tiles[c][:],
            )
    # u0 MLP (reference-head weights, feat at k=0)
    ft0 = p_w1.tile([6, 512], F32R, tag="ft0")
    nc.sync.dma_start(out=ft0[:], in_=ft_d[bc, :, 0, :])
    z0 = p_zps.tile([128, 1024], F32, tag="z")
    nc.tensor.matmul(z0[:, 0:512], w1r[:], ft0[:], start=True, stop=True)
    h0 = p_h.tile([128, 1024], F16, tag="h")
    nc.scalar.activation(h0[:, 0:512], z0[:, 0:512], AF.Tanh)
    ou = p_ops.tile([128, 384], F32, tag="ops")
    for c in range(4):
        nc.tensor.matmul(ou[:, c * 2:(c + 1) * 2], h0[:, c * 128:(c + 1) * 128],
                         w2r[:], start=True, stop=True)
    nc.vector.tensor_copy(U0[:, bc * 8:(bc + 1) * 8], ou[:, 0:8])


def _phase_b(nc, tc, p_tmp, bc, pl, oall, Sr, Si):
    """Elementwise combine + step reduction for one 512-batch chunk."""
    o4 = oall[:].rearrange("p (c k j) -> p c k j", c=4, k=NK)
    og_r, og_i = o4[:, :, :, 0:3], o4[:, :, :, 3:6]
    oi_r, oi_i = o4[:, :, :, 6], o4[:, :, :, 7]
    op_r, op_i = o4[:, :, :, 8], o4[:, :, :, 9]
    om_r, om_i = o4[:, :, :, 10], o4[:, :, :, 11]
    xp = pl[:, _XP:_XP + 1536].rearrange("p (c k i) -> p c k i", c=4, k=NK)
    dbp = pl[:, _DBP:_DBP + 1536].rearrange("p (c k i) -> p c k i", c=4, k=NK)
    E1 = pl[:, _E1:_E1 + 512]
    E2 = pl[:, _E2:_E2 + 512]
    CUP = pl[:, _CUP:_CUP + 512]
    CUM = pl[:, _CUM:_CUM + 512]
    QP = pl[:, _QP:_QP + 512]
    EFPR = pl[:, _EFPR:_EFPR + 512]
    EFPI = pl[:, _EFPI:_EFPI + 512]
    DCR = pl[:, _DCR:_DCR + 512]
    DCI = pl[:, _DCI:_DCI + 512]

    def T(name, cols=512):
        return p_tmp.tile([128, cols], F32, tag=name, name=name)[:]

    prod = p_tmp.tile([128, 1536], F32, tag="prod", name="prod")[:]
    prod4 = prod.rearrange("p (c k i) -> p c k i", c=4, k=NK)

    v = nc.vector
    d1r, d1i, d2r, d2i = T("d1r"), T("d1i"), T("d2r"), T("d2i")
    sgr, sgi = T("sgr"), T("sgi")
    dupr, dupi, dumr, dumi = T("dupr"), T("dupi"), T("dumr"), T("dumi")
    deltr, delti = T("deltr"), T("delti")
    apr, api = T("apr"), T("api")
    t1 = T("t1")

    for (dst, a, b_) in ((d1r, og_r, dbp), (d1i, og_i, dbp),
                         (d2r, og_r, xp), (d2i, og_i, xp)):
        v.tensor_mul(prod4, a, b_)
        v.reduce_sum(dst, prod4, axis=AX.X)
    v.reduce_sum(sgr, og_r, axis=AX.X)
    v.reduce_sum(sgi, og_i, axis=AX.X)
    v.tensor_sub(dupr, op_r, oi_r)
    v.tensor_sub(dupi, op_i, oi_i)
    v.tensor_sub(dumr, om_r, oi_r)
    v.tensor_sub(dumi, om_i, oi_i)

    for (dst, dd1, dd2, dup, dum, sg, dc) in (
            (deltr, d1r, d2r, dupr, dumr, sgr, DCR),
            (delti, d1i, d2i, dupi, dumi, sgi, DCI)):
        v.tensor_mul(dst, E1, dd1)
        v.tensor_mul(t1, E2, dd2)
        v.tensor_sub(dst, dst, t1)
        v.tensor_mul(t1, CUP, dup)
        v.tensor_add(dst, dst, t1)
        v.tensor_mul(t1, CUM, dum)
        v.tensor_add(dst, dst, t1)
        v.tensor_mul(t1, QP, sg)
        v.tensor_sub(dst, dst, t1)
        v.tensor_add(dst, dst, dc)

    v.tensor_mul(apr, EFPR, deltr)
    v.tensor_mul(t1, EFPI, delti)
    v.tensor_sub(apr, apr, t1)
    v.tensor_mul(api, EFPR, delti)
    v.tensor_mul(t1, EFPI, deltr)
    v.tensor_add(api, api, t1)

    v.reduce_sum(Sr[:, bc * 4:(bc + 1) * 4],
                 apr.rearrange("p (c k) -> p c k", c=4), axis=AX.X)
    v.reduce_sum(Si[:, bc * 4:(bc + 1) * 4],
                 api.rearrange("p (c k) -> p c k", c=4), axis=AX.X)


def _kernel_body(ctx, tc, ft_d, w1_d, w2_d, w1r_d, w2r_d, pl_d, fin_d, u_d, g_d,
                 repeats=1):
    nc = tc.nc
    p_const = ctx.enter_context(tc.tile_pool(name="const", bufs=1))
    p_ft = ctx.enter_context(tc.tile_pool(name="ftp", bufs=2))
    p_w1 = ctx.enter_context(tc.tile_pool(name="w1p", bufs=2))
    p_h = ctx.enter_context(tc.tile_pool(name="hp", bufs=2))
    p_oall = ctx.enter_context(tc.tile_pool(name="oallp", bufs=2))
    p_pl = ctx.enter_context(tc.tile_pool(name="plp", bufs=1))
    p_tmp = ctx.enter_context(tc.tile_pool(name="tmpp", bufs=1))
    p_zps = ctx.enter_context(tc.tile_pool(name="zpsp", bufs=2, space="PSUM"))
    p_ops = ctx.enter_context(tc.tile_pool(name="opsp", bufs=4, space="PSUM"))

    w2all = p_const.tile([128, NK * 12], F16)
    nc.sync.dma_start(out=w2all[:].rearrange("p (k j) -> p k j", k=NK), in_=w2_d)
    w1r = p_const.tile([6, 128], F32R)
    nc.sync.dma_start(out=w1r[:], in_=w1r_d)
    w2r = p_const.tile([128, 2], F16)
    nc.sync.dma_start(out=w2r[:], in_=w2r_d)
    fin = p_const.tile([128, 192], F32)
    nc.sync.dma_start(out=fin[:], in_=fin_d)

    Sr = p_const.tile([128, 4 * NBC], F32)
    Si = p_const.tile([128, 4 * NBC], F32)
    U0 = p_const.tile([128, 8 * NBC], F32)

    mlp_pools = (p_ft, p_w1, p_h, p_zps, p_ops)
    for bc in [b for _ in range(repeats) for b in range(NBC)]:
        pl = p_pl.tile([128, PL_COLS], F32, tag="pl")
        nc.gpsimd.dma_start(out=pl[:], in_=pl_d[bc])
        oall_t = p_oall.tile([128, 6144], F16, tag="oall")
        _phase_a(nc, tc, mlp_pools, bc, ft_d, w1_d, w2all, w1r, w2r, oall_t[:], U0)
        _phase_b(nc, tc, p_tmp, bc, pl[:], oall_t[:], Sr[:], Si[:])

    # final assembly
    v = nc.vector
    PF = fin[:, 0:32]
    E8R = fin[:, 32:64]
    E8I = fin[:, 64:96]
    X1f, X2f, X3f = fin[:, 96:128], fin[:, 128:160], fin[:, 160:192]
    outu = p_const.tile([128, 8 * NBC], F32)
    outg = p_const.tile([128, 8 * NBC], F32)
    xs = p_tmp.tile([128, 32], F32, tag="fxs", name="fxs")[:]
    tu = p_tmp.tile([128, 32], F32, tag="ftu", name="ftu")[:]
    outu_v = outu[:].rearrange("p (b r) -> p b r", r=2)
    outg_v = outg[:].rearrange("p (b r) -> p b r", r=2)
    u0_v = U0[:].rearrange("p (b r) -> p b r", r=2)

    v.tensor_add(xs, X1f, X2f)
    v.tensor_add(xs, xs, X3f)
    v.tensor_mul(outg_v[:, :, 0], E8R, xs)
    v.tensor_mul(outg_v[:, :, 1], E8I, xs)
    v.tensor_mul(tu, u0_v[:, :, 0], PF)
    v.tensor_add(outu_v[:, :, 0], tu, Sr[:])
    v.tensor_mul(tu, u0_v[:, :, 1], PF)
    v.tensor_add(outu_v[:, :, 1], tu, Si[:])

    nc.sync.dma_start(
        out=u_d.rearrange("(bc c bp) ri -> bp bc c ri", bc=NBC, c=4),
        in_=outu[:].rearrange("p (bc c ri) -> p bc c ri", bc=NBC, c=4),
    )
    nc.sync.dma_start(
        out=g_d.rearrange("(bc c bp) ri -> bp bc c ri", bc=NBC, c=4),
        in_=outg[:].rearrange("p (bc c ri) -> p bc c ri", bc=NBC, c=4),
    )


def build_nc(repeats=1):
    nc = bacc.Bacc("TRN2", target_bir_lowering=False, debug=False)
    ft_d = nc.dram_tensor("ft", [NBC, 6, NK, BC], F32R, kind="ExternalInput").ap()
    w1_d = nc.dram_tensor("w1", [NK, 6, 256], F32R, kind="ExternalInput").ap()
    w2_d = nc.dram_tensor("w2", [128, NK, 12], F16, kind="ExternalInput").ap()
    w1r_d = nc.dram_tensor("w1r", [6, 128], F32R, kind="ExternalInput").ap()
    w2r_d = nc.dram_tensor("w2r", [128, 2], F16, kind="ExternalInput").ap()
    pl_d = nc.dram_tensor("planes", [NBC, 128, PL_COLS], F32, kind="ExternalInput").ap()
    fin_d = nc.dram_tensor("fin", [128, 192], F32, kind="ExternalInput").ap()
    u_d = nc.dram_tensor("u_ri", [B_LOC, 2], F32, kind="ExternalOutput").ap()
    g_d = nc.dram_tensor("g_ri", [B_LOC, 2], F32, kind="ExternalOutput").ap()
    with tile.TileContext(nc) as tc:
        with ExitStack() as ctx:
            _kernel_body(ctx, tc, ft_d, w1_d, w2_d, w1r_d, w2r_d, pl_d, fin_d,
                         u_d, g_d, repeats=repeats)
    nc.compile()
    return nc


# ----------------------------------------------------------------------------
# host-side preparation
# ----------------------------------------------------------------------------

def _to_bck(gc):
    """[128k, 4096b] -> [8bc, 128bp, 4c, 128k] for one core."""
    return gc.reshape(NK, NBC, 4, 128).transpose(1, 3, 2, 0)


def prep_host(inp):
    f32, f64 = np.float32, np.float64
    N = np.asarray(inp["process_N"], f32)[:, :, 0]
    X = np.asarray(inp["process_X"], f32)
    P = np.asarray(inp["discrete_p"], f32)[:, :, 0]
    T = np.asarray(inp["discrete_t"], f32)
    dB = np.asarray(inp["delta_B"], f32)

    n, x, p, t = N[:NK], X[:NK], P[:NK], T[:NK]
    dN = np.round(N[1:] - N[:NK])

    s = np.sum(x * x, axis=-1)
    theta = (p * s).astype(f64)
    phi = (DT_STEP * (np.cumsum(theta, axis=0) - theta)).astype(f64)
    efr = np.cos(phi).astype(f32)
    efi = (-np.sin(phi)).astype(f32)

    kD = np.sqrt(1.0 + 0.2 * np.abs(n))
    m0 = (dN == 0).astype(f32)
    mp_ = (dN > 0).astype(f32)
    mm_ = (dN < 0).astype(f32)
    w2c = 0.4 / (1.0 + s)
    d3 = np.sum(x * dB, axis=-1)
    E1 = m0 * kD * np.float32(0.5)
    E2 = m0 * kD * w2c * d3
    CUP = mp_ - m0 * (0.5 * (n + 1.0)) * np.float32(DT_STEP)
    CUM = mm_ - m0 * (0.4 * np.abs(n) + 0.1) * np.float32(DT_STEP)
    QP = m0 * (np.float32(0.1 * DT_STEP) * (1.0 + t[:, None]))
    c = (1.0 - m0 * p * np.float32(DT_STEP)).astype(f64)
    SP = np.ones_like(c)
    SP[:-1] = np.cumprod(c[::-1], axis=0)[::-1][1:]
    Pfull = (c[0] * SP[0]).astype(f32)
    EFPR = (efr * SP).astype(f32)
    EFPI = (efi * SP).astype(f32)

    phi128 = DT_STEP * np.cumsum(theta, axis=0)[-1]
    EF128R = np.cos(phi128).astype(f32)
    EF128I = (-np.sin(phi128)).astype(f32)

    # weights (shared across cores)
    Wg1, bg1 = np.asarray(inp["Wg1"], f32), np.asarray(inp["bg1"], f32)
    Wg2 = np.asarray(inp["Wg2"], f32)
    bg2 = np.asarray(inp["bg2"], f32)
    Wj1, bj1 = np.asarray(inp["Wj1"], f32), np.asarray(inp["bj1"], f32)
    Wj2 = np.asarray(inp["Wj2"], f32)
    bj2 = np.asarray(inp["bj2"], f32)
    Wr1, br1 = np.asarray(inp["Wr1"], f32), np.asarray(inp["br1"], f32)
    Wr2, br2 = np.asarray(inp["Wr2"], f32), np.asarray(inp["br2"], f32)
    w0 = Wj1[:, 0]

    w1a = np.zeros((NK, 6, 128), f32)
    w1a[:, 0:5, 0:64] = Wg1
    w1a[:, 5, 0:64] = bg1
    w1a[:, 0:5, 64:128] = Wj1
    w1a[:, 5, 64:128] = bj1
    w1b = np.zeros((NK, 6, 128), f32)
    w1b[:, 0:5, 0:64] = Wj1
    w1b[:, 5, 0:64] = bj1 + w0
    w1b[:, 0:5, 64:128] = Wj1
    w1b[:, 5, 64:128] = bj1 - w0
    w1_host = np.ascontiguousarray(np.concatenate([w1a, w1b], axis=2))

    # The device layer-2 omits the output biases: dup/dum cancel bj2 exactly,
    # and the constant bg2 contribution to delt is folded into the additive
    # host planes DCR/DCI below.  br2 (u0 head) is re-added on the host.
    w2cat = np.zeros((NK, 128, 12), f32)
    w2cat[:, 0:64, 0:6] = Wg2
    w2cat[:, 64:128, 6:8] = Wj2
    w2cat[:, 0:64, 8:10] = Wj2
    w2cat[:, 64:128, 10:12] = Wj2
    w2_host = np.ascontiguousarray(w2cat.transpose(1, 0, 2)).astype(np.float16)

    w1r_host = np.zeros((6, 128), f32)
    w1r_host[0:5, 0:64] = Wr1
    w1r_host[5, 0:64] = br1
    w2r_host = np.zeros((128, 2), np.float16)
    w2r_host[0:64] = Wr2.astype(np.float16)

    # --- fold constant layer-2 biases into the coefficient planes ---
    # o_g = o_g_dev + bg2  (bg2 = [br(3), bi(3)])
    # o_i/o_p/o_m get + bj2, which cancels in dup/dum.
    # delt uses: E1*(gu.dB) - E2*(gu.x) - QP*sum(gu)  with gu = gu_dev + bg2.
    # Additive correction (real):  E1*(bgr.dB) - E2*(bgr.x) - QP*sum(bgr)
    # This is an input-only plane; append to EFP-side as a delt offset:
    #   deltr_true = deltr_dev + DCR ,  delti_true = delti_dev + DCI
    # Then a' = EFP * delt: fold DCR/DCI into S on the host?  S is a device
    # reduction of EFP*delt; the correction sum_k EFP_k*DC_k is fully
    # host-computable, so add it to u via the final combine: we fold it into
    # the PF/S path by adding the correction to Sr/Si through... the device
    # adds outu = u0*PF + S; host cannot inject there.  We instead fold DC
    # into the plane pair (EFPR, EFPI) is impossible (multiplicative).
    # => device-side: deltr starts as E1*d1 (dev) ... we add one more fused
    # add using a 14th/15th plane pair DCR/DCI.
    bgr, bgi = bg2[:, 0:3], bg2[:, 3:6]
    DCR = (E1 * np.einsum("kj,kbj->kb", bgr, dB)
           - E2 * np.einsum("kj,kbj->kb", bgr, x)
           - QP * bgr.sum(axis=1)[:, None])
    DCI = (E1 * np.einsum("kj,kbj->kb", bgi, dB)
           - E2 * np.einsum("kj,kbj->kb", bgi, x)
           - QP * bgi.sum(axis=1)[:, None])

    in_maps = []
    for ci in range(N_CORES):
        sl = slice(ci * B_LOC, (ci + 1) * B_LOC)
        ftc = np.stack([n[:, sl], x[:, sl, 0], x[:, sl, 1], x[:, sl, 2],
                        p[:, sl], np.ones_like(p[:, sl])], axis=1)  # [128,6,4096]
        ft_host = np.ascontiguousarray(
            ftc.reshape(NK, 6, NBC, BC).transpose(2, 1, 0, 3))

        xpc = X[:NK, sl].reshape(NK, NBC, 4, 128, 3).transpose(1, 3, 2, 0, 4)
        dbc = dB[:, sl].reshape(NK, NBC, 4, 128, 3).transpose(1, 3, 2, 0, 4)
        singles = [_to_bck(a[:, sl]) for a in
                   (E1, E2, CUP, CUM, QP, EFPR, EFPI, DCR, DCI)]
        pl_host = np.concatenate(
            [xpc.reshape(NBC, 128, 1536), dbc.reshape(NBC, 128, 1536)]
            + [a.reshape(NBC, 128, 512) for a in singles], axis=2)
        pl_host = np.ascontiguousarray(pl_host, dtype=f32)

        def fincol(a):
            return a[sl].reshape(NBC, 4, 128).transpose(2, 0, 1).reshape(128, 32)

        fin_host = np.ascontiguousarray(np.concatenate(
            [fincol(Pfull), fincol(EF128R), fincol(EF128I),
             fincol(X[NK, :, 0]), fincol(X[NK, :, 1]), fincol(X[NK, :, 2])],
            axis=1), dtype=f32)

        in_maps.append({
            "ft": ft_host, "w1": w1_host, "w2": w2_host,
            "w1r": w1r_host, "w2r": w2r_host,
            "planes": pl_host, "fin": fin_host,
        })
    return in_maps, Pfull, br2


_NC_CACHE = {}


def kernel(**inputs):
    in_maps, Pfull, br2 = prep_host(inputs)
    if "nc" not in _NC_CACHE:
        _NC_CACHE["nc"] = build_nc()
    nc = _NC_CACHE["nc"]
    res = run_bass_kernel_spmd(nc, in_maps, list(range(N_CORES)))
    u_parts, g_parts = [], []
    for ci in range(N_CORES):
        ur = res.results[ci]["u_ri"]
        gr = res.results[ci]["g_ri"]
        u_parts.append(ur[:, 0] + 1j * ur[:, 1])
        g_parts.append(gr[:, 0] + 1j * gr[:, 1])
    u = np.concatenate(u_parts)
    # fold the u0 layer-2 bias (br2, constant) back in: u += (br2_r+i br2_i)*Pfull
    u = u + (br2[0] + 1j * br2[1]) * Pfull.astype(np.float64)
    g = np.concatenate(g_parts)
    u = u.astype(np.complex64)[:, None]
    g = g.astype(np.complex64)[:, None]
    return u, g


# revision 17
# speedup vs baseline: 34.3958x; 1.0047x over previous
"""Trainium2 Bass kernel for nn_DeepBSDESC (DeepBSDE forward pass).

Strategy
--------
The reference scan over 128 time steps is *affine* in the carried state u:
    u_{k+1} = c_k * u_k + a_k
where c_k (real) and a_k (complex) do not depend on u.  Hence
    u_final = (prod_k c_k) * u0 + sum_k a_k * prod_{j>k} c_j
and every step's a_k can be evaluated independently (no sequential loop on
device).  The 3x3 matrix algebra collapses analytically:
    T_inv @ sigma0^T = 0.5*I - 0.4*x x^T / (1+|x|^2)
so grad_bmm reduces to dot products.

Sharding: data-parallel over batch B=32768 across 8 cores (4096 each), MLP
weights replicated.  Host precomputes input-only coefficient planes (masks,
suffix products, exp-functional phases); the device evaluates all MLPs
(>99% of FLOPs) and the per-step combine, then reduces over steps.

Device pipeline per core, per 512-batch chunk, per step k:
  L1  : z = [ft @ (Wg1|Wj1) + bias ; ft @ (Wj1|Wj1) +- w0-shifted bias]
        via two f32r matmuls (bias through an appended ones-feature row)
  tanh: one ACT pass [128,1024] PSUM->SBUF fp16
  L2  : transposed matmuls (H chunk stationary, small weight matrix moving)
        -> batch-major outputs accumulated in PSUM over 32 steps
  combine: DVE elementwise with host coefficient planes, reduce over steps.
"""

import os
import sys

import numpy as np

for _p in ("/opt/trn_rl_repo", "/root/.axon_site/_ro/trn_rl_repo"):
    if os.path.isdir(_p) and _p not in sys.path:
        sys.path.append(_p)

from contextlib import ExitStack

import concourse.bass as bass
import concourse.bacc as bacc
import concourse.tile as tile
from concourse import mybir
from concourse.bass_utils import run_bass_kernel_spmd

N_CORES = 8
NK = 128                 # time steps
B_FULL = 32768
B_LOC = B_FULL // N_CORES  # 4096
NBC = 8                  # 512-batch chunks per core
BC = 512
DT_STEP = 1.0 / NK

F32 = mybir.dt.float32
F32R = mybir.dt.float32r
F16 = mybir.dt.float16
AF = mybir.ActivationFunctionType
AX = mybir.AxisListType

# plane blob column offsets (per 512-batch chunk, [128, 7680])
_XP, _DBP = 0, 1536
_E1, _E2, _CUP, _CUM, _QP, _EFPR, _EFPI = 3072, 3584, 4096, 4608, 5120, 5632, 6144
_DCR, _DCI = 6656, 7168
PL_COLS = 7680


def _phase_a(nc, tc, pools, bc, ft_d, w1_d, w2all, w1r, w2r, oall, U0,
             pb_chunks=None):
    """MLP evaluation for one 512-batch chunk, all 128 steps.

    After each 32-step group's drain, emits one chunk of the PREVIOUS batch
    chunk's phase-B ops so the DVE queue interleaves drains with combine work
    (a monolithic phase-B emission would make each drain wait behind ~44 us
    of queued DVE ops, stalling the in-order PE queue and starving ACT).
    """
    p_ft, p_w1, p_h, p_zps, p_ops = pools
    ftt = None
    for kg in range(4):
        if pb_chunks:
            pb_chunks[kg]()
        ops_tiles = [p_ops.tile([128, 384], F32, tag="ops", name=f"ops{c}")
                     for c in range(4)]
        for kk in range(32):
            k = kg * 32 + kk
            if k % 16 == 0:
                w1t = p_w1.tile([6, 16 * 256], F32R, tag="w1")
                nc.sync.dma_start(
                    out=w1t[:].rearrange("p (a b) -> p a b", a=16),
                    in_=w1_d[k:k + 16].rearrange("a p b -> p a b"),
                )
            if k % 8 == 0:
                ftt = p_ft.tile([6, 8 * BC], F32R, tag="ft")
                nc.sync.dma_start(
                    out=ftt[:].rearrange("p (a b) -> p a b", a=8),
                    in_=ft_d[bc, :, k:k + 8, :],
                )
            zt = p_zps.tile([128, 1024], F32, tag="z")
            rhs = ftt[:, (k % 8) * BC:(k % 8 + 1) * BC]
            kw = kk % 16
            nc.tensor.matmul(zt[:, 0:512], w1t[:, kw * 256:kw * 256 + 128], rhs,
                             start=True, stop=True)
            nc.tensor.matmul(zt[:, 512:1024], w1t[:, kw * 256 + 128:kw * 256 + 256],
                             rhs, start=True, stop=True)
            ht = p_h.tile([128, 1024], F16, tag="h")
            nc.scalar.activation(ht[:], zt[:], AF.Tanh)
            for c in range(4):
                nc.tensor.matmul(
                    ops_tiles[c][:, kk * 12:kk * 12 + 8],
                    ht[:, c * 128:(c + 1) * 128],
                    w2all[:, k * 12:k * 12 + 8],
                    start=True, stop=True,
                )
                nc.tensor.matmul(
                    ops_tiles[c][:, kk * 12 + 8:kk * 12 + 12],
                    ht[:, 512 + c * 128:512 + (c + 1) * 128],
                    w2all[:, k * 12 + 8:k * 12 + 12],
                    start=True, stop=True,
                )
        for c in range(4):
            nc.vector.tensor_copy(
                oall[:, c * 1536 + kg * 384:c * 1536 + (kg + 1) * 384],
                ops_tiles[c][:],
            )


def _u0_all(nc, pools, ft_d, w1r, w2r, U0):
    """u0 head for all batch chunks, batched at the end."""
    p_ft, p_w1, p_h, p_zps, p_ops = pools
    ft0 = p_ft.tile([6, 8 * BC], F32R, tag="ft")
    nc.sync.dma_start(out=ft0[:].rearrange("p (a b) -> p a b", a=NBC),
                      in_=ft_d[:, :, 0, :].rearrange("a p b -> p a b"))
    for bc2 in range(0, NBC, 2):
        z0 = p_zps.tile([128, 1024], F32, tag="z")
        nc.tensor.matmul(z0[:, 0:512], w1r[:],
                         ft0[:, bc2 * BC:(bc2 + 1) * BC], start=True, stop=True)
        nc.tensor.matmul(z0[:, 512:1024], w1r[:],
                         ft0[:, (bc2 + 1) * BC:(bc2 + 2) * BC], start=True, stop=True)
        h0 = p_h.tile([128, 1024], F16, tag="h")
        nc.scalar.activation(h0[:], z0[:], AF.Tanh)
        ou = p_ops.tile([128, 384], F32, tag="ops", name="ou")
        for half in range(2):
            for c in range(4):
                nc.tensor.matmul(
                    ou[:, half * 8 + c * 2:half * 8 + (c + 1) * 2],
                    h0[:, half * 512 + c * 128:half * 512 + (c + 1) * 128],
                    w2r[:], start=True, stop=True)
        nc.vector.tensor_copy(U0[:, bc2 * 8:(bc2 + 2) * 8], ou[:, 0:16])


def _phase_b_chunks(nc, tc, p_tmp, bc, pl, oall, Sr, Si):
    """Elementwise combine + step reduction for one 512-batch chunk.

    Returns 4 emitters (deferred op groups) for interleaved emission."""
    o4 = oall[:].rearrange("p (c k j) -> p c k j", c=4, k=NK)
    og_r, og_i = o4[:, :, :, 0:3], o4[:, :, :, 3:6]
    oi_r, oi_i = o4[:, :, :, 6], o4[:, :, :, 7]
    op_r, op_i = o4[:, :, :, 8], o4[:, :, :, 9]
    om_r, om_i = o4[:, :, :, 10], o4[:, :, :, 11]
    xp = pl[:, _XP:_XP + 1536].rearrange("p (c k i) -> p c k i", c=4, k=NK)
    dbp = pl[:, _DBP:_DBP + 1536].rearrange("p (c k i) -> p c k i", c=4, k=NK)
    E1 = pl[:, _E1:_E1 + 512]
    E2 = pl[:, _E2:_E2 + 512]
    CUP = pl[:, _CUP:_CUP + 512]
    CUM = pl[:, _CUM:_CUM + 512]
    QP = pl[:, _QP:_QP + 512]
    EFPR = pl[:, _EFPR:_EFPR + 512]
    EFPI = pl[:, _EFPI:_EFPI + 512]
    DCR = pl[:, _DCR:_DCR + 512]
    DCI = pl[:, _DCI:_DCI + 512]

    def T(name, cols=512):
        return p_tmp.tile([128, cols], F32, tag=name, name=name)[:]

    prod = p_tmp.tile([128, 1536], F32, tag="prod", name="prod")[:]
    prod4 = prod.rearrange("p (c k i) -> p c k i", c=4, k=NK)

    v = nc.vector
    d1r, d1i, d2r, d2i = T("d1r"), T("d1i"), T("d2r"), T("d2i")
    sgr, sgi = T("sgr"), T("sgi")
    dupr, dupi, dumr, dumi = T("dupr"), T("dupi"), T("dumr"), T("dumi")
    deltr, delti = T("deltr"), T("delti")
    apr, api = T("apr"), T("api")
    t1 = T("t1")

    def chunk0():
        for (dst, a, b_) in ((d1r, og_r, dbp), (d1i, og_i, dbp),
                             (d2r, og_r, xp), (d2i, og_i, xp)):
            v.tensor_mul(prod4, a, b_)
            v.reduce_sum(dst, prod4, axis=AX.X)

    def chunk1():
        v.reduce_sum(sgr, og_r, axis=AX.X)
        v.reduce_sum(sgi, og_i, axis=AX.X)
        v.tensor_sub(dupr, op_r, oi_r)
        v.tensor_sub(dupi, op_i, oi_i)
        v.tensor_sub(dumr, om_r, oi_r)
        v.tensor_sub(dumi, om_i, oi_i)

    def _delt(dst, dd1, dd2, dup, dum, sg, dc):
        v.tensor_mul(dst, E1, dd1)
        v.tensor_mul(t1, E2, dd2)
        v.tensor_sub(dst, dst, t1)
        v.tensor_mul(t1, CUP, dup)
        v.tensor_add(dst, dst, t1)
        v.tensor_mul(t1, CUM, dum)
        v.tensor_add(dst, dst, t1)
        v.tensor_mul(t1, QP, sg)
        v.tensor_sub(dst, dst, t1)
        v.tensor_add(dst, dst, dc)

    def chunk2():
        _delt(deltr, d1r, d2r, dupr, dumr, sgr, DCR)

    def chunk3():
        _delt(delti, d1i, d2i, dupi, dumi, sgi, DCI)
        v.tensor_mul(apr, EFPR, deltr)
        v.tensor_mul(t1, EFPI, delti)
        v.tensor_sub(apr, apr, t1)
        v.tensor_mul(api, EFPR, delti)
        v.tensor_mul(t1, EFPI, deltr)
        v.tensor_add(api, api, t1)
        v.reduce_sum(Sr[:, bc * 4:(bc + 1) * 4],
                     apr.rearrange("p (c k) -> p c k", c=4), axis=AX.X)
        v.reduce_sum(Si[:, bc * 4:(bc + 1) * 4],
                     api.rearrange("p (c k) -> p c k", c=4), axis=AX.X)

    return [chunk0, chunk1, chunk2, chunk3]


def _kernel_body(ctx, tc, ft_d, w1_d, w2_d, w1r_d, w2r_d, pl_d, fin_d, u_d, g_d,
                 repeats=1):
    nc = tc.nc
    p_const = ctx.enter_context(tc.tile_pool(name="const", bufs=1))
    p_ft = ctx.enter_context(tc.tile_pool(name="ftp", bufs=2))
    p_w1 = ctx.enter_context(tc.tile_pool(name="w1p", bufs=2))
    p_h = ctx.enter_context(tc.tile_pool(name="hp", bufs=3))
    p_oall = ctx.enter_context(tc.tile_pool(name="oallp", bufs=2))
    p_pl = ctx.enter_context(tc.tile_pool(name="plp", bufs=1))
    p_tmp = ctx.enter_context(tc.tile_pool(name="tmpp", bufs=1))
    p_zps = ctx.enter_context(tc.tile_pool(name="zpsp", bufs=2, space="PSUM"))
    p_ops = ctx.enter_context(tc.tile_pool(name="opsp", bufs=4, space="PSUM"))

    w2all = p_const.tile([128, NK * 12], F16)
    nc.sync.dma_start(out=w2all[:].rearrange("p (k j) -> p k j", k=NK), in_=w2_d)
    w1r = p_const.tile([6, 128], F32R)
    nc.sync.dma_start(out=w1r[:], in_=w1r_d)
    w2r = p_const.tile([128, 2], F16)
    nc.sync.dma_start(out=w2r[:], in_=w2r_d)
    fin = p_const.tile([128, 192], F32)
    nc.sync.dma_start(out=fin[:], in_=fin_d)

    Sr = p_const.tile([128, 4 * NBC], F32)
    Si = p_const.tile([128, 4 * NBC], F32)
    U0 = p_const.tile([128, 8 * NBC], F32)

    mlp_pools = (p_ft, p_w1, p_h, p_zps, p_ops)
    pb_chunks = None
    for bc in [b for _ in range(repeats) for b in range(NBC)]:
        pl = p_pl.tile([128, PL_COLS], F32, tag="pl")
        nc.gpsimd.dma_start(out=pl[:], in_=pl_d[bc])
        oall_t = p_oall.tile([128, 6144], F16, tag="oall")
        _phase_a(nc, tc, mlp_pools, bc, ft_d, w1_d, w2all, w1r, w2r, oall_t[:], U0,
                 pb_chunks=pb_chunks)
        pb_chunks = _phase_b_chunks(nc, tc, p_tmp, bc, pl[:], oall_t[:], Sr[:], Si[:])

    for ch in pb_chunks:
        ch()
    _u0_all(nc, mlp_pools, ft_d, w1r, w2r, U0)

    # final assembly
    v = nc.vector
    PF = fin[:, 0:32]
    E8R = fin[:, 32:64]
    E8I = fin[:, 64:96]
    X1f, X2f, X3f = fin[:, 96:128], fin[:, 128:160], fin[:, 160:192]
    outu = p_const.tile([128, 8 * NBC], F32)
    outg = p_const.tile([128, 8 * NBC], F32)
    xs = p_tmp.tile([128, 32], F32, tag="fxs", name="fxs")[:]
    tu = p_tmp.tile([128, 32], F32, tag="ftu", name="ftu")[:]
    outu_v = outu[:].rearrange("p (b r) -> p b r", r=2)
    outg_v = outg[:].rearrange("p (b r) -> p b r", r=2)
    u0_v = U0[:].rearrange("p (b r) -> p b r", r=2)

    v.tensor_add(xs, X1f, X2f)
    v.tensor_add(xs, xs, X3f)
    v.tensor_mul(outg_v[:, :, 0], E8R, xs)
    v.tensor_mul(outg_v[:, :, 1], E8I, xs)
    v.tensor_mul(tu, u0_v[:, :, 0], PF)
    v.tensor_add(outu_v[:, :, 0], tu, Sr[:])
    v.tensor_mul(tu, u0_v[:, :, 1], PF)
    v.tensor_add(outu_v[:, :, 1], tu, Si[:])

    nc.sync.dma_start(
        out=u_d.rearrange("(bc c bp) ri -> bp bc c ri", bc=NBC, c=4),
        in_=outu[:].rearrange("p (bc c ri) -> p bc c ri", bc=NBC, c=4),
    )
    nc.sync.dma_start(
        out=g_d.rearrange("(bc c bp) ri -> bp bc c ri", bc=NBC, c=4),
        in_=outg[:].rearrange("p (bc c ri) -> p bc c ri", bc=NBC, c=4),
    )


def build_nc(repeats=1):
    nc = bacc.Bacc("TRN2", target_bir_lowering=False, debug=False)
    ft_d = nc.dram_tensor("ft", [NBC, 6, NK, BC], F32R, kind="ExternalInput").ap()
    w1_d = nc.dram_tensor("w1", [NK, 6, 256], F32R, kind="ExternalInput").ap()
    w2_d = nc.dram_tensor("w2", [128, NK, 12], F16, kind="ExternalInput").ap()
    w1r_d = nc.dram_tensor("w1r", [6, 128], F32R, kind="ExternalInput").ap()
    w2r_d = nc.dram_tensor("w2r", [128, 2], F16, kind="ExternalInput").ap()
    pl_d = nc.dram_tensor("planes", [NBC, 128, PL_COLS], F32, kind="ExternalInput").ap()
    fin_d = nc.dram_tensor("fin", [128, 192], F32, kind="ExternalInput").ap()
    u_d = nc.dram_tensor("u_ri", [B_LOC, 2], F32, kind="ExternalOutput").ap()
    g_d = nc.dram_tensor("g_ri", [B_LOC, 2], F32, kind="ExternalOutput").ap()
    with tile.TileContext(nc) as tc:
        with ExitStack() as ctx:
            _kernel_body(ctx, tc, ft_d, w1_d, w2_d, w1r_d, w2r_d, pl_d, fin_d,
                         u_d, g_d, repeats=repeats)
    nc.compile()
    return nc


# ----------------------------------------------------------------------------
# host-side preparation
# ----------------------------------------------------------------------------

def _to_bck(gc):
    """[128k, 4096b] -> [8bc, 128bp, 4c, 128k] for one core."""
    return gc.reshape(NK, NBC, 4, 128).transpose(1, 3, 2, 0)


def prep_host(inp):
    f32, f64 = np.float32, np.float64
    N = np.asarray(inp["process_N"], f32)[:, :, 0]
    X = np.asarray(inp["process_X"], f32)
    P = np.asarray(inp["discrete_p"], f32)[:, :, 0]
    T = np.asarray(inp["discrete_t"], f32)
    dB = np.asarray(inp["delta_B"], f32)

    n, x, p, t = N[:NK], X[:NK], P[:NK], T[:NK]
    dN = np.round(N[1:] - N[:NK])

    s = np.sum(x * x, axis=-1)
    theta = (p * s).astype(f64)
    phi = (DT_STEP * (np.cumsum(theta, axis=0) - theta)).astype(f64)
    efr = np.cos(phi).astype(f32)
    efi = (-np.sin(phi)).astype(f32)

    kD = np.sqrt(1.0 + 0.2 * np.abs(n))
    m0 = (dN == 0).astype(f32)
    mp_ = (dN > 0).astype(f32)
    mm_ = (dN < 0).astype(f32)
    w2c = 0.4 / (1.0 + s)
    d3 = np.sum(x * dB, axis=-1)
    E1 = m0 * kD * np.float32(0.5)
    E2 = m0 * kD * w2c * d3
    CUP = mp_ - m0 * (0.5 * (n + 1.0)) * np.float32(DT_STEP)
    CUM = mm_ - m0 * (0.4 * np.abs(n) + 0.1) * np.float32(DT_STEP)
    QP = m0 * (np.float32(0.1 * DT_STEP) * (1.0 + t[:, None]))
    c = (1.0 - m0 * p * np.float32(DT_STEP)).astype(f64)
    SP = np.ones_like(c)
    SP[:-1] = np.cumprod(c[::-1], axis=0)[::-1][1:]
    Pfull = (c[0] * SP[0]).astype(f32)
    EFPR = (efr * SP).astype(f32)
    EFPI = (efi * SP).astype(f32)

    phi128 = DT_STEP * np.cumsum(theta, axis=0)[-1]
    EF128R = np.cos(phi128).astype(f32)
    EF128I = (-np.sin(phi128)).astype(f32)

    # weights (shared across cores)
    Wg1, bg1 = np.asarray(inp["Wg1"], f32), np.asarray(inp["bg1"], f32)
    Wg2 = np.asarray(inp["Wg2"], f32)
    bg2 = np.asarray(inp["bg2"], f32)
    Wj1, bj1 = np.asarray(inp["Wj1"], f32), np.asarray(inp["bj1"], f32)
    Wj2 = np.asarray(inp["Wj2"], f32)
    bj2 = np.asarray(inp["bj2"], f32)
    Wr1, br1 = np.asarray(inp["Wr1"], f32), np.asarray(inp["br1"], f32)
    Wr2, br2 = np.asarray(inp["Wr2"], f32), np.asarray(inp["br2"], f32)
    w0 = Wj1[:, 0]

    w1a = np.zeros((NK, 6, 128), f32)
    w1a[:, 0:5, 0:64] = Wg1
    w1a[:, 5, 0:64] = bg1
    w1a[:, 0:5, 64:128] = Wj1
    w1a[:, 5, 64:128] = bj1
    w1b = np.zeros((NK, 6, 128), f32)
    w1b[:, 0:5, 0:64] = Wj1
    w1b[:, 5, 0:64] = bj1 + w0
    w1b[:, 0:5, 64:128] = Wj1
    w1b[:, 5, 64:128] = bj1 - w0
    w1_host = np.ascontiguousarray(np.concatenate([w1a, w1b], axis=2))

    # The device layer-2 omits the output biases: dup/dum cancel bj2 exactly,
    # and the constant bg2 contribution to delt is folded into the additive
    # host planes DCR/DCI below.  br2 (u0 head) is re-added on the host.
    w2cat = np.zeros((NK, 128, 12), f32)
    w2cat[:, 0:64, 0:6] = Wg2
    w2cat[:, 64:128, 6:8] = Wj2
    w2cat[:, 0:64, 8:10] = Wj2
    w2cat[:, 64:128, 10:12] = Wj2
    w2_host = np.ascontiguousarray(w2cat.transpose(1, 0, 2)).astype(np.float16)

    w1r_host = np.zeros((6, 128), f32)
    w1r_host[0:5, 0:64] = Wr1
    w1r_host[5, 0:64] = br1
    w2r_host = np.zeros((128, 2), np.float16)
    w2r_host[0:64] = Wr2.astype(np.float16)

    # --- fold constant layer-2 biases into the coefficient planes ---
    # o_g = o_g_dev + bg2  (bg2 = [br(3), bi(3)])
    # o_i/o_p/o_m get + bj2, which cancels in dup/dum.
    # delt uses: E1*(gu.dB) - E2*(gu.x) - QP*sum(gu)  with gu = gu_dev + bg2.
    # Additive correction (real):  E1*(bgr.dB) - E2*(bgr.x) - QP*sum(bgr)
    # This is an input-only plane; append to EFP-side as a delt offset:
    #   deltr_true = deltr_dev + DCR ,  delti_true = delti_dev + DCI
    # Then a' = EFP * delt: fold DCR/DCI into S on the host?  S is a device
    # reduction of EFP*delt; the correction sum_k EFP_k*DC_k is fully
    # host-computable, so add it to u via the final combine: we fold it into
    # the PF/S path by adding the correction to Sr/Si through... the device
    # adds outu = u0*PF + S; host cannot inject there.  We instead fold DC
    # into the plane pair (EFPR, EFPI) is impossible (multiplicative).
    # => device-side: deltr starts as E1*d1 (dev) ... we add one more fused
    # add using a 14th/15th plane pair DCR/DCI.
    bgr, bgi = bg2[:, 0:3], bg2[:, 3:6]
    DCR = (E1 * np.einsum("kj,kbj->kb", bgr, dB)
           - E2 * np.einsum("kj,kbj->kb", bgr, x)
           - QP * bgr.sum(axis=1)[:, None])
    DCI = (E1 * np.einsum("kj,kbj->kb", bgi, dB)
           - E2 * np.einsum("kj,kbj->kb", bgi, x)
           - QP * bgi.sum(axis=1)[:, None])

    in_maps = []
    for ci in range(N_CORES):
        sl = slice(ci * B_LOC, (ci + 1) * B_LOC)
        ftc = np.stack([n[:, sl], x[:, sl, 0], x[:, sl, 1], x[:, sl, 2],
                        p[:, sl], np.ones_like(p[:, sl])], axis=1)  # [128,6,4096]
        ft_host = np.ascontiguousarray(
            ftc.reshape(NK, 6, NBC, BC).transpose(2, 1, 0, 3))

        xpc = X[:NK, sl].reshape(NK, NBC, 4, 128, 3).transpose(1, 3, 2, 0, 4)
        dbc = dB[:, sl].reshape(NK, NBC, 4, 128, 3).transpose(1, 3, 2, 0, 4)
        singles = [_to_bck(a[:, sl]) for a in
                   (E1, E2, CUP, CUM, QP, EFPR, EFPI, DCR, DCI)]
        pl_host = np.concatenate(
            [xpc.reshape(NBC, 128, 1536), dbc.reshape(NBC, 128, 1536)]
            + [a.reshape(NBC, 128, 512) for a in singles], axis=2)
        pl_host = np.ascontiguousarray(pl_host, dtype=f32)

        def fincol(a):
            return a[sl].reshape(NBC, 4, 128).transpose(2, 0, 1).reshape(128, 32)

        fin_host = np.ascontiguousarray(np.concatenate(
            [fincol(Pfull), fincol(EF128R), fincol(EF128I),
             fincol(X[NK, :, 0]), fincol(X[NK, :, 1]), fincol(X[NK, :, 2])],
            axis=1), dtype=f32)

        in_maps.append({
            "ft": ft_host, "w1": w1_host, "w2": w2_host,
            "w1r": w1r_host, "w2r": w2r_host,
            "planes": pl_host, "fin": fin_host,
        })
    return in_maps, Pfull, br2


_NC_CACHE = {}


def kernel(**inputs):
    in_maps, Pfull, br2 = prep_host(inputs)
    if "nc" not in _NC_CACHE:
        _NC_CACHE["nc"] = build_nc()
    nc = _NC_CACHE["nc"]
    res = run_bass_kernel_spmd(nc, in_maps, list(range(N_CORES)))
    u_parts, g_parts = [], []
    for ci in range(N_CORES):
        ur = res.results[ci]["u_ri"]
        gr = res.results[ci]["g_ri"]
        u_parts.append(ur[:, 0] + 1j * ur[:, 1])
        g_parts.append(gr[:, 0] + 1j * gr[:, 1])
    u = np.concatenate(u_parts)
    # fold the u0 layer-2 bias (br2, constant) back in: u += (br2_r+i br2_i)*Pfull
    u = u + (br2[0] + 1j * br2[1]) * Pfull.astype(np.float64)
    g = np.concatenate(g_parts)
    u = u.astype(np.complex64)[:, None]
    g = g.astype(np.complex64)[:, None]
    return u, g


# revision 18
# speedup vs baseline: 47.8612x; 1.3915x over previous
"""Trainium2 Bass kernel for nn_DeepBSDESC (DeepBSDE forward pass).

Strategy
--------
The reference scan over 128 time steps is *affine* in the carried state u:
    u_{k+1} = c_k * u_k + a_k
where c_k (real) and a_k (complex) do not depend on u.  Hence
    u_final = (prod_k c_k) * u0 + sum_k a_k * prod_{j>k} c_j
and every step's a_k can be evaluated independently (no sequential loop on
device).  The 3x3 matrix algebra collapses analytically:
    T_inv @ sigma0^T = 0.5*I - 0.4*x x^T / (1+|x|^2)
so grad_bmm reduces to dot products.

Sharding: data-parallel over batch B=32768 across 8 cores (4096 each), MLP
weights replicated.  Host precomputes input-only coefficient planes (masks,
suffix products, exp-functional phases); the device evaluates all MLPs
(>99% of FLOPs) and the per-step combine, then reduces over steps.

Device pipeline per core, per 512-batch chunk, per step k:
  L1  : z = [ft @ (Wg1|Wj1) + bias ; ft @ (Wj1|Wj1) +- w0-shifted bias]
        via two f32r matmuls (bias through an appended ones-feature row)
  tanh: one ACT pass [128,1024] PSUM->SBUF fp16
  L2  : transposed matmuls (H chunk stationary, small weight matrix moving)
        -> batch-major outputs accumulated in PSUM over 32 steps
  combine: DVE elementwise with host coefficient planes, reduce over steps.
"""

import os
import sys

import numpy as np

for _p in ("/opt/trn_rl_repo", "/root/.axon_site/_ro/trn_rl_repo"):
    if os.path.isdir(_p) and _p not in sys.path:
        sys.path.append(_p)

from contextlib import ExitStack

import concourse.bass as bass
import concourse.bacc as bacc
import concourse.tile as tile
from concourse import mybir
from concourse.bass_utils import run_bass_kernel_spmd

N_CORES = 8
NK = 128                 # time steps
B_FULL = 32768
B_LOC = B_FULL // N_CORES  # 4096
NBC = 8                  # 512-batch chunks per core
BC = 512
DT_STEP = 1.0 / NK

F32 = mybir.dt.float32
F32R = mybir.dt.float32r
F16 = mybir.dt.float16
AF = mybir.ActivationFunctionType
AX = mybir.AxisListType

# plane blob column offsets (per 512-batch chunk, [128, 7680])
_XP, _DBP = 0, 1536
_E1, _E2, _CUP, _CUM, _QP, _EFPR, _EFPI = 3072, 3584, 4096, 4608, 5120, 5632, 6144
_DCR, _DCI = 6656, 7168
PL_COLS = 7680


def _phase_a(nc, tc, pools, bc, ft_d, w1_d, w2all, w1r, w2r, oall, U0,
             pb_chunks=None):
    """MLP evaluation for one 512-batch chunk, all 128 steps.

    After each 32-step group's drain, emits one chunk of the PREVIOUS batch
    chunk's phase-B ops so the DVE queue interleaves drains with combine work
    (a monolithic phase-B emission would make each drain wait behind ~44 us
    of queued DVE ops, stalling the in-order PE queue and starving ACT).
    """
    p_ft, p_w1, p_h, p_zps, p_ops = pools
    ftt = None
    ops_tiles = {}
    pending = []

    def emit_l2(k2, ht2):
        # L2 matmuls are emitted 2 steps behind L1/tanh: at a 32-step group
        # boundary the first L2 of the new group waits on the PSUM drains, and
        # the in-order PE queue would otherwise stall the next steps' L1s
        # behind it, starving ACT.  The delay keeps 2 L1/tanh steps ahead of
        # any drain-stalled L2.
        kg2, kk2 = k2 // 32, k2 % 32
        if kg2 not in ops_tiles:
            ops_tiles[kg2] = [p_ops.tile([128, 384], F32, tag="ops",
                                         name=f"ops{kg2}_{c}") for c in range(4)]
        for c in range(4):
            nc.tensor.matmul(
                ops_tiles[kg2][c][:, kk2 * 12:kk2 * 12 + 8],
                ht2[:, c * 128:(c + 1) * 128],
                w2all[:, k2 * 12:k2 * 12 + 8],
                start=True, stop=True,
            )
            nc.tensor.matmul(
                ops_tiles[kg2][c][:, kk2 * 12 + 8:kk2 * 12 + 12],
                ht2[:, 512 + c * 128:512 + (c + 1) * 128],
                w2all[:, k2 * 12 + 8:k2 * 12 + 12],
                start=True, stop=True,
            )
        if kk2 == 31:
            for c in range(4):
                nc.vector.tensor_copy(
                    oall[:, c * 1536 + kg2 * 384:c * 1536 + (kg2 + 1) * 384],
                    ops_tiles[kg2][c][:],
                )
            del ops_tiles[kg2]

    for k in range(NK):
        if k % 32 == 0 and pb_chunks:
            pb_chunks[k // 32]()
        if k % 16 == 0:
            w1t = p_w1.tile([6, 16 * 256], F32R, tag="w1")
            nc.sync.dma_start(
                out=w1t[:].rearrange("p (a b) -> p a b", a=16),
                in_=w1_d[k:k + 16].rearrange("a p b -> p a b"),
            )
        if k % 8 == 0:
            ftt = p_ft.tile([6, 8 * BC], F32R, tag="ft")
            nc.sync.dma_start(
                out=ftt[:].rearrange("p (a b) -> p a b", a=8),
                in_=ft_d[bc, :, k:k + 8, :],
            )
        zt = p_zps.tile([128, 1024], F32, tag="z")
        rhs = ftt[:, (k % 8) * BC:(k % 8 + 1) * BC]
        kw = k % 16
        nc.tensor.matmul(zt[:, 0:512], w1t[:, kw * 256:kw * 256 + 128], rhs,
                         start=True, stop=True)
        nc.tensor.matmul(zt[:, 512:1024], w1t[:, kw * 256 + 128:kw * 256 + 256],
                         rhs, start=True, stop=True)
        ht = p_h.tile([128, 1024], F16, tag="h")
        nc.scalar.activation(ht[:], zt[:], AF.Tanh)
        pending.append((k, ht))
        if len(pending) > 2:
            emit_l2(*pending.pop(0))
    while pending:
        emit_l2(*pending.pop(0))


def _u0_all(nc, pools, ft_d, w1r, w2r, U0):
    """u0 head for all batch chunks, batched at the end."""
    p_ft, p_w1, p_h, p_zps, p_ops = pools
    ft0 = p_ft.tile([6, 8 * BC], F32R, tag="ft")
    nc.sync.dma_start(out=ft0[:].rearrange("p (a b) -> p a b", a=NBC),
                      in_=ft_d[:, :, 0, :].rearrange("a p b -> p a b"))
    for bc2 in range(0, NBC, 2):
        z0 = p_zps.tile([128, 1024], F32, tag="z")
        nc.tensor.matmul(z0[:, 0:512], w1r[:],
                         ft0[:, bc2 * BC:(bc2 + 1) * BC], start=True, stop=True)
        nc.tensor.matmul(z0[:, 512:1024], w1r[:],
                         ft0[:, (bc2 + 1) * BC:(bc2 + 2) * BC], start=True, stop=True)
        h0 = p_h.tile([128, 1024], F16, tag="h")
        nc.scalar.activation(h0[:], z0[:], AF.Tanh)
        ou = p_ops.tile([128, 384], F32, tag="ops", name="ou")
        for half in range(2):
            for c in range(4):
                nc.tensor.matmul(
                    ou[:, half * 8 + c * 2:half * 8 + (c + 1) * 2],
                    h0[:, half * 512 + c * 128:half * 512 + (c + 1) * 128],
                    w2r[:], start=True, stop=True)
        nc.vector.tensor_copy(U0[:, bc2 * 8:(bc2 + 2) * 8], ou[:, 0:16])


def _phase_b_chunks(nc, tc, p_tmp, bc, pl, oall, Sr, Si):
    """Elementwise combine + step reduction for one 512-batch chunk.

    Returns 4 emitters (deferred op groups) for interleaved emission."""
    o4 = oall[:].rearrange("p (c k j) -> p c k j", c=4, k=NK)
    og_r, og_i = o4[:, :, :, 0:3], o4[:, :, :, 3:6]
    oi_r, oi_i = o4[:, :, :, 6], o4[:, :, :, 7]
    op_r, op_i = o4[:, :, :, 8], o4[:, :, :, 9]
    om_r, om_i = o4[:, :, :, 10], o4[:, :, :, 11]
    xp = pl[:, _XP:_XP + 1536].rearrange("p (c k i) -> p c k i", c=4, k=NK)
    dbp = pl[:, _DBP:_DBP + 1536].rearrange("p (c k i) -> p c k i", c=4, k=NK)
    E1 = pl[:, _E1:_E1 + 512]
    E2 = pl[:, _E2:_E2 + 512]
    CUP = pl[:, _CUP:_CUP + 512]
    CUM = pl[:, _CUM:_CUM + 512]
    QP = pl[:, _QP:_QP + 512]
    EFPR = pl[:, _EFPR:_EFPR + 512]
    EFPI = pl[:, _EFPI:_EFPI + 512]
    DCR = pl[:, _DCR:_DCR + 512]
    DCI = pl[:, _DCI:_DCI + 512]

    def T(name, cols=512):
        return p_tmp.tile([128, cols], F32, tag=name, name=name)[:]

    prod = p_tmp.tile([128, 1536], F32, tag="prod", name="prod")[:]
    prod4 = prod.rearrange("p (c k i) -> p c k i", c=4, k=NK)

    v = nc.vector
    d1r, d1i, d2r, d2i = T("d1r"), T("d1i"), T("d2r"), T("d2i")
    sgr, sgi = T("sgr"), T("sgi")
    dupr, dupi, dumr, dumi = T("dupr"), T("dupi"), T("dumr"), T("dumi")
    deltr, delti = T("deltr"), T("delti")
    apr, api = T("apr"), T("api")
    t1 = T("t1")

    def chunk0():
        for (dst, a, b_) in ((d1r, og_r, dbp), (d1i, og_i, dbp),
                             (d2r, og_r, xp), (d2i, og_i, xp)):
            v.tensor_mul(prod4, a, b_)
            v.reduce_sum(dst, prod4, axis=AX.X)

    def chunk1():
        v.reduce_sum(sgr, og_r, axis=AX.X)
        v.reduce_sum(sgi, og_i, axis=AX.X)
        v.tensor_sub(dupr, op_r, oi_r)
        v.tensor_sub(dupi, op_i, oi_i)
        v.tensor_sub(dumr, om_r, oi_r)
        v.tensor_sub(dumi, om_i, oi_i)

    def _delt(dst, dd1, dd2, dup, dum, sg, dc):
        v.tensor_mul(dst, E1, dd1)
        v.tensor_mul(t1, E2, dd2)
        v.tensor_sub(dst, dst, t1)
        v.tensor_mul(t1, CUP, dup)
        v.tensor_add(dst, dst, t1)
        v.tensor_mul(t1, CUM, dum)
        v.tensor_add(dst, dst, t1)
        v.tensor_mul(t1, QP, sg)
        v.tensor_sub(dst, dst, t1)
        v.tensor_add(dst, dst, dc)

    def chunk2():
        _delt(deltr, d1r, d2r, dupr, dumr, sgr, DCR)

    def chunk3():
        _delt(delti, d1i, d2i, dupi, dumi, sgi, DCI)
        v.tensor_mul(apr, EFPR, deltr)
        v.tensor_mul(t1, EFPI, delti)
        v.tensor_sub(apr, apr, t1)
        v.tensor_mul(api, EFPR, delti)
        v.tensor_mul(t1, EFPI, deltr)
        v.tensor_add(api, api, t1)
        v.reduce_sum(Sr[:, bc * 4:(bc + 1) * 4],
                     apr.rearrange("p (c k) -> p c k", c=4), axis=AX.X)
        v.reduce_sum(Si[:, bc * 4:(bc + 1) * 4],
                     api.rearrange("p (c k) -> p c k", c=4), axis=AX.X)

    return [chunk0, chunk1, chunk2, chunk3]


def _kernel_body(ctx, tc, ft_d, w1_d, w2_d, w1r_d, w2r_d, pl_d, fin_d, u_d, g_d,
                 repeats=1):
    nc = tc.nc
    p_const = ctx.enter_context(tc.tile_pool(name="const", bufs=1))
    p_ft = ctx.enter_context(tc.tile_pool(name="ftp", bufs=2))
    p_w1 = ctx.enter_context(tc.tile_pool(name="w1p", bufs=2))
    p_h = ctx.enter_context(tc.tile_pool(name="hp", bufs=4))
    p_oall = ctx.enter_context(tc.tile_pool(name="oallp", bufs=2))
    p_pl = ctx.enter_context(tc.tile_pool(name="plp", bufs=1))
    p_tmp = ctx.enter_context(tc.tile_pool(name="tmpp", bufs=1))
    p_zps = ctx.enter_context(tc.tile_pool(name="zpsp", bufs=2, space="PSUM"))
    p_ops = ctx.enter_context(tc.tile_pool(name="opsp", bufs=4, space="PSUM"))

    w2all = p_const.tile([128, NK * 12], F16)
    nc.sync.dma_start(out=w2all[:].rearrange("p (k j) -> p k j", k=NK), in_=w2_d)
    w1r = p_const.tile([6, 128], F32R)
    nc.sync.dma_start(out=w1r[:], in_=w1r_d)
    w2r = p_const.tile([128, 2], F16)
    nc.sync.dma_start(out=w2r[:], in_=w2r_d)
    fin = p_const.tile([128, 192], F32)
    nc.sync.dma_start(out=fin[:], in_=fin_d)

    Sr = p_const.tile([128, 4 * NBC], F32)
    Si = p_const.tile([128, 4 * NBC], F32)
    U0 = p_const.tile([128, 8 * NBC], F32)

    mlp_pools = (p_ft, p_w1, p_h, p_zps, p_ops)
    pb_chunks = None
    for bc in [b for _ in range(repeats) for b in range(NBC)]:
        pl = p_pl.tile([128, PL_COLS], F32, tag="pl")
        nc.gpsimd.dma_start(out=pl[:], in_=pl_d[bc])
        oall_t = p_oall.tile([128, 6144], F16, tag="oall")
        _phase_a(nc, tc, mlp_pools, bc, ft_d, w1_d, w2all, w1r, w2r, oall_t[:], U0,
                 pb_chunks=pb_chunks)
        pb_chunks = _phase_b_chunks(nc, tc, p_tmp, bc, pl[:], oall_t[:], Sr[:], Si[:])

    for ch in pb_chunks:
        ch()
    _u0_all(nc, mlp_pools, ft_d, w1r, w2r, U0)

    # final assembly
    v = nc.vector
    PF = fin[:, 0:32]
    E8R = fin[:, 32:64]
    E8I = fin[:, 64:96]
    X1f, X2f, X3f = fin[:, 96:128], fin[:, 128:160], fin[:, 160:192]
    outu = p_const.tile([128, 8 * NBC], F32)
    outg = p_const.tile([128, 8 * NBC], F32)
    xs = p_tmp.tile([128, 32], F32, tag="fxs", name="fxs")[:]
    tu = p_tmp.tile([128, 32], F32, tag="ftu", name="ftu")[:]
    outu_v = outu[:].rearrange("p (b r) -> p b r", r=2)
    outg_v = outg[:].rearrange("p (b r) -> p b r", r=2)
    u0_v = U0[:].rearrange("p (b r) -> p b r", r=2)

    v.tensor_add(xs, X1f, X2f)
    v.tensor_add(xs, xs, X3f)
    v.tensor_mul(outg_v[:, :, 0], E8R, xs)
    v.tensor_mul(outg_v[:, :, 1], E8I, xs)
    v.tensor_mul(tu, u0_v[:, :, 0], PF)
    v.tensor_add(outu_v[:, :, 0], tu, Sr[:])
    v.tensor_mul(tu, u0_v[:, :, 1], PF)
    v.tensor_add(outu_v[:, :, 1], tu, Si[:])

    nc.sync.dma_start(
        out=u_d.rearrange("(bc c bp) ri -> bp bc c ri", bc=NBC, c=4),
        in_=outu[:].rearrange("p (bc c ri) -> p bc c ri", bc=NBC, c=4),
    )
    nc.sync.dma_start(
        out=g_d.rearrange("(bc c bp) ri -> bp bc c ri", bc=NBC, c=4),
        in_=outg[:].rearrange("p (bc c ri) -> p bc c ri", bc=NBC, c=4),
    )


def build_nc(repeats=1):
    nc = bacc.Bacc("TRN2", target_bir_lowering=False, debug=False)
    ft_d = nc.dram_tensor("ft", [NBC, 6, NK, BC], F32R, kind="ExternalInput").ap()
    w1_d = nc.dram_tensor("w1", [NK, 6, 256], F32R, kind="ExternalInput").ap()
    w2_d = nc.dram_tensor("w2", [128, NK, 12], F16, kind="ExternalInput").ap()
    w1r_d = nc.dram_tensor("w1r", [6, 128], F32R, kind="ExternalInput").ap()
    w2r_d = nc.dram_tensor("w2r", [128, 2], F16, kind="ExternalInput").ap()
    pl_d = nc.dram_tensor("planes", [NBC, 128, PL_COLS], F32, kind="ExternalInput").ap()
    fin_d = nc.dram_tensor("fin", [128, 192], F32, kind="ExternalInput").ap()
    u_d = nc.dram_tensor("u_ri", [B_LOC, 2], F32, kind="ExternalOutput").ap()
    g_d = nc.dram_tensor("g_ri", [B_LOC, 2], F32, kind="ExternalOutput").ap()
    with tile.TileContext(nc) as tc:
        with ExitStack() as ctx:
            _kernel_body(ctx, tc, ft_d, w1_d, w2_d, w1r_d, w2r_d, pl_d, fin_d,
                         u_d, g_d, repeats=repeats)
    nc.compile()
    return nc


# ----------------------------------------------------------------------------
# host-side preparation
# ----------------------------------------------------------------------------

def _to_bck(gc):
    """[128k, 4096b] -> [8bc, 128bp, 4c, 128k] for one core."""
    return gc.reshape(NK, NBC, 4, 128).transpose(1, 3, 2, 0)


def prep_host(inp):
    f32, f64 = np.float32, np.float64
    N = np.asarray(inp["process_N"], f32)[:, :, 0]
    X = np.asarray(inp["process_X"], f32)
    P = np.asarray(inp["discrete_p"], f32)[:, :, 0]
    T = np.asarray(inp["discrete_t"], f32)
    dB = np.asarray(inp["delta_B"], f32)

    n, x, p, t = N[:NK], X[:NK], P[:NK], T[:NK]
    dN = np.round(N[1:] - N[:NK])

    s = np.sum(x * x, axis=-1)
    theta = (p * s).astype(f64)
    phi = (DT_STEP * (np.cumsum(theta, axis=0) - theta)).astype(f64)
    efr = np.cos(phi).astype(f32)
    efi = (-np.sin(phi)).astype(f32)

    kD = np.sqrt(1.0 + 0.2 * np.abs(n))
    m0 = (dN == 0).astype(f32)
    mp_ = (dN > 0).astype(f32)
    mm_ = (dN < 0).astype(f32)
    w2c = 0.4 / (1.0 + s)
    d3 = np.sum(x * dB, axis=-1)
    E1 = m0 * kD * np.float32(0.5)
    E2 = m0 * kD * w2c * d3
    CUP = mp_ - m0 * (0.5 * (n + 1.0)) * np.float32(DT_STEP)
    CUM = mm_ - m0 * (0.4 * np.abs(n) + 0.1) * np.float32(DT_STEP)
    QP = m0 * (np.float32(0.1 * DT_STEP) * (1.0 + t[:, None]))
    c = (1.0 - m0 * p * np.float32(DT_STEP)).astype(f64)
    SP = np.ones_like(c)
    SP[:-1] = np.cumprod(c[::-1], axis=0)[::-1][1:]
    Pfull = (c[0] * SP[0]).astype(f32)
    EFPR = (efr * SP).astype(f32)
    EFPI = (efi * SP).astype(f32)

    phi128 = DT_STEP * np.cumsum(theta, axis=0)[-1]
    EF128R = np.cos(phi128).astype(f32)
    EF128I = (-np.sin(phi128)).astype(f32)

    # weights (shared across cores)
    Wg1, bg1 = np.asarray(inp["Wg1"], f32), np.asarray(inp["bg1"], f32)
    Wg2 = np.asarray(inp["Wg2"], f32)
    bg2 = np.asarray(inp["bg2"], f32)
    Wj1, bj1 = np.asarray(inp["Wj1"], f32), np.asarray(inp["bj1"], f32)
    Wj2 = np.asarray(inp["Wj2"], f32)
    bj2 = np.asarray(inp["bj2"], f32)
    Wr1, br1 = np.asarray(inp["Wr1"], f32), np.asarray(inp["br1"], f32)
    Wr2, br2 = np.asarray(inp["Wr2"], f32), np.asarray(inp["br2"], f32)
    w0 = Wj1[:, 0]

    w1a = np.zeros((NK, 6, 128), f32)
    w1a[:, 0:5, 0:64] = Wg1
    w1a[:, 5, 0:64] = bg1
    w1a[:, 0:5, 64:128] = Wj1
    w1a[:, 5, 64:128] = bj1
    w1b = np.zeros((NK, 6, 128), f32)
    w1b[:, 0:5, 0:64] = Wj1
    w1b[:, 5, 0:64] = bj1 + w0
    w1b[:, 0:5, 64:128] = Wj1
    w1b[:, 5, 64:128] = bj1 - w0
    w1_host = np.ascontiguousarray(np.concatenate([w1a, w1b], axis=2))

    # The device layer-2 omits the output biases: dup/dum cancel bj2 exactly,
    # and the constant bg2 contribution to delt is folded into the additive
    # host planes DCR/DCI below.  br2 (u0 head) is re-added on the host.
    w2cat = np.zeros((NK, 128, 12), f32)
    w2cat[:, 0:64, 0:6] = Wg2
    w2cat[:, 64:128, 6:8] = Wj2
    w2cat[:, 0:64, 8:10] = Wj2
    w2cat[:, 64:128, 10:12] = Wj2
    w2_host = np.ascontiguousarray(w2cat.transpose(1, 0, 2)).astype(np.float16)

    w1r_host = np.zeros((6, 128), f32)
    w1r_host[0:5, 0:64] = Wr1
    w1r_host[5, 0:64] = br1
    w2r_host = np.zeros((128, 2), np.float16)
    w2r_host[0:64] = Wr2.astype(np.float16)

    # --- fold constant layer-2 biases into the coefficient planes ---
    # o_g = o_g_dev + bg2  (bg2 = [br(3), bi(3)])
    # o_i/o_p/o_m get + bj2, which cancels in dup/dum.
    # delt uses: E1*(gu.dB) - E2*(gu.x) - QP*sum(gu)  with gu = gu_dev + bg2.
    # Additive correction (real):  E1*(bgr.dB) - E2*(bgr.x) - QP*sum(bgr)
    # This is an input-only plane; append to EFP-side as a delt offset:
    #   deltr_true = deltr_dev + DCR ,  delti_true = delti_dev + DCI
    # Then a' = EFP * delt: fold DCR/DCI into S on the host?  S is a device
    # reduction of EFP*delt; the correction sum_k EFP_k*DC_k is fully
    # host-computable, so add it to u via the final combine: we fold it into
    # the PF/S path by adding the correction to Sr/Si through... the device
    # adds outu = u0*PF + S; host cannot inject there.  We instead fold DC
    # into the plane pair (EFPR, EFPI) is impossible (multiplicative).
    # => device-side: deltr starts as E1*d1 (dev) ... we add one more fused
    # add using a 14th/15th plane pair DCR/DCI.
    bgr, bgi = bg2[:, 0:3], bg2[:, 3:6]
    DCR = (E1 * np.einsum("kj,kbj->kb", bgr, dB)
           - E2 * np.einsum("kj,kbj->kb", bgr, x)
           - QP * bgr.sum(axis=1)[:, None])
    DCI = (E1 * np.einsum("kj,kbj->kb", bgi, dB)
           - E2 * np.einsum("kj,kbj->kb", bgi, x)
           - QP * bgi.sum(axis=1)[:, None])

    in_maps = []
    for ci in range(N_CORES):
        sl = slice(ci * B_LOC, (ci + 1) * B_LOC)
        ftc = np.stack([n[:, sl], x[:, sl, 0], x[:, sl, 1], x[:, sl, 2],
                        p[:, sl], np.ones_like(p[:, sl])], axis=1)  # [128,6,4096]
        ft_host = np.ascontiguousarray(
            ftc.reshape(NK, 6, NBC, BC).transpose(2, 1, 0, 3))

        xpc = X[:NK, sl].reshape(NK, NBC, 4, 128, 3).transpose(1, 3, 2, 0, 4)
        dbc = dB[:, sl].reshape(NK, NBC, 4, 128, 3).transpose(1, 3, 2, 0, 4)
        singles = [_to_bck(a[:, sl]) for a in
                   (E1, E2, CUP, CUM, QP, EFPR, EFPI, DCR, DCI)]
        pl_host = np.concatenate(
            [xpc.reshape(NBC, 128, 1536), dbc.reshape(NBC, 128, 1536)]
            + [a.reshape(NBC, 128, 512) for a in singles], axis=2)
        pl_host = np.ascontiguousarray(pl_host, dtype=f32)

        def fincol(a):
            return a[sl].reshape(NBC, 4, 128).transpose(2, 0, 1).reshape(128, 32)

        fin_host = np.ascontiguousarray(np.concatenate(
            [fincol(Pfull), fincol(EF128R), fincol(EF128I),
             fincol(X[NK, :, 0]), fincol(X[NK, :, 1]), fincol(X[NK, :, 2])],
            axis=1), dtype=f32)

        in_maps.append({
            "ft": ft_host, "w1": w1_host, "w2": w2_host,
            "w1r": w1r_host, "w2r": w2r_host,
            "planes": pl_host, "fin": fin_host,
        })
    return in_maps, Pfull, br2


_NC_CACHE = {}


def kernel(**inputs):
    in_maps, Pfull, br2 = prep_host(inputs)
    if "nc" not in _NC_CACHE:
        _NC_CACHE["nc"] = build_nc()
    nc = _NC_CACHE["nc"]
    res = run_bass_kernel_spmd(nc, in_maps, list(range(N_CORES)))
    u_parts, g_parts = [], []
    for ci in range(N_CORES):
        ur = res.results[ci]["u_ri"]
        gr = res.results[ci]["g_ri"]
        u_parts.append(ur[:, 0] + 1j * ur[:, 1])
        g_parts.append(gr[:, 0] + 1j * gr[:, 1])
    u = np.concatenate(u_parts)
    # fold the u0 layer-2 bias (br2, constant) back in: u += (br2_r+i br2_i)*Pfull
    u = u + (br2[0] + 1j * br2[1]) * Pfull.astype(np.float64)
    g = np.concatenate(g_parts)
    u = u.astype(np.complex64)[:, None]
    g = g.astype(np.complex64)[:, None]
    return u, g
